# revision 28
# baseline (speedup 1.0000x reference)
"""Causal self-attention (RoPE) Trainium2 kernel, 8-way head-parallel.

Sharding: each of the 8 cores computes 2 of the 16 heads for all 4 batches
(tensor parallel over heads: W_qkv column-split, W_proj row-split). Host
pre-transposes x -> xT [C, B*T], slices per-core weights, and sum-reduces the
8 partial projection outputs (+ b_proj) — the standard row-parallel TP reduce.

Per-core dataflow (bf16 storage/matmuls, fp32 PSUM):
  qkvT = W_slice.T @ xT            [feat, tok] PSUM, bias added on ACT evac
  RoPE on qT,kT                    (rotate-half via permutation matmul on PE)
  v: PE-transpose vT -> vc tiles   [tok, 2*(HD+1)] (+ ones col per head)
  per (b, h, i-chunk, jg of 2 j-tiles):
      S^T = kT_jt.T @ qT_ichunk    (j on partitions)
      P^T = exp(S^T/8) (ACT), causal mask on diagonal tiles (mult, DVE)
      [yT_h | denom] += vc_h.T @ P^T   accumulated over j-tiles in PSUM
  yT_h *= recip(denom) (approx recip, bf16 bcast), out = sum_h yT_h.T @ Wp_h

The emission schedule software-pipelines at j-group granularity: between a
j-group's S matmuls and the PREVIOUS group's AV matmuls we pop one "filler"
unit (a QKV weight-stream for batch b+1 or a projection half for an earlier
chunk) so the PE stream stays dense while ACT computes exp.
"""

from collections import deque

import numpy as np

import concourse.bass as bass
import concourse.mybir as mybir
import concourse.tile as tile

F32 = mybir.dt.float32
BF16 = mybir.dt.bfloat16
AF = mybir.ActivationFunctionType
OP = mybir.AluOpType

# ---------------------------------------------------------------- tile patch
# This walrus build rejects >1 embedded sync-wait on sync-engine CTRL
# instructions; Tile's tail drain embeds one wait per outstanding semaphore.
# Split them across NOPs (1 wait each) before the drain.


def _patched_drain_and_barrier(self, tick_clock, wait_clock):
    from concourse.tile import ScopedClock

    nc = self.nc
    probe = nc.sync.nop(nofuse=True)
    wait_clock.add_sem_waits(probe.ins, ScopedClock({None: tick_clock.global_clock}))
    si = probe.ins.sync_info
    waits = list(si.on_wait) if si is not None and si.on_wait else []
    if len(waits) > 1:
        si.on_wait = waits[:1]
        for w in waits[1:]:
            nop = nc.sync.nop(nofuse=True)
            nsi = nop.ins.sync_info
            if nsi is None:
                nop.ins.sync_info = mybir.SyncInfo(on_wait=[w], on_update=[])
            else:
                nsi.on_wait = [w]
    nc.sync.drain()
    nc.all_engine_barrier()
    assert self.sems is not None
    popped = nc._tile_sem_poison_stack.pop()
    assert popped is self._sem_poison
    # chunk the sem clears: the range-encoded gpsimd drain (dma_reset) in this
    # walrus build rejects wide semaphore ranges ("ISA wrong length")
    sems = sorted(
        s.num if hasattr(s, "num") else s for s in self.sems.allocated().values()
    )
    for i in range(0, len(sems), 16):
        nc.clear_and_free_semaphores(sems[i : i + 16])
    nc.all_engine_barrier()


tile.TileContext._drain_and_barrier = _patched_drain_and_barrier


def _split_waits(nc):
    """Hoist all but one sync-wait per instruction onto same-engine NOPs
    (this walrus codegen supports a single embedded wait per instruction)."""
    n = 0
    for f in nc.m.functions:
        for blk in f.blocks:
            out = []
            changed = False
            for ins in blk.instructions:
                si = ins.sync_info
                if si is not None and si.on_wait and len(si.on_wait) > 1:
                    waits = list(si.on_wait)
                    for w in waits[:-1]:
                        n += 1
                        out.append(
                            mybir.InstNoOp(
                                name=f"wsplit{n}",
                                engine=ins.engine,
                                sync_info=mybir.SyncInfo(on_wait=[w], on_update=[]),
                                bass_nofuse=True,
                            )
                        )
                    si.on_wait = waits[-1:]
                    changed = True
                out.append(ins)
            if changed:
                blk.instructions = out
    return n


# ------------------------------------------------------------------- builder

HD = 64  # head dim (fixed)
ROPE_BASE = 10000.0


def build_nc(B, T, C, split=True):
    """One core's program: 2 heads x B batches. T % 512 == 0, C % 128 == 0."""
    assert T % 512 == 0 and C % 128 == 0
    TOK = B * T
    KC = C // 128   # contraction chunks for QKV
    NCH = T // 512  # i-chunks per batch
    NJT = T // 128  # j-tiles per batch
    FW = 128        # qkv feature width per tensor (2 heads * 64)
    HALF = NCH // 2

    nc = bass.Bass()
    xT = nc.dram_tensor("xT", [C, TOK], BF16, kind="ExternalInput")
    wq = nc.dram_tensor("wq", [C, FW], BF16, kind="ExternalInput")
    wk = nc.dram_tensor("wk", [C, FW], BF16, kind="ExternalInput")
    wv = nc.dram_tensor("wv", [C, FW], BF16, kind="ExternalInput")
    bq = nc.dram_tensor("bq", [FW, 1], F32, kind="ExternalInput")
    bk = nc.dram_tensor("bk", [FW, 1], F32, kind="ExternalInput")
    bv = nc.dram_tensor("bv", [FW, 1], F32, kind="ExternalInput")
    wp = nc.dram_tensor("wp", [FW, C], BF16, kind="ExternalInput")
    cosT = nc.dram_tensor("cosT", [FW, T], BF16, kind="ExternalInput")
    sinT = nc.dram_tensor("sinT", [FW, T], BF16, kind="ExternalInput")
    perm = nc.dram_tensor("perm", [128, 128], BF16, kind="ExternalInput")
    mask4 = nc.dram_tensor("mask4", [128, 4 * 512], BF16, kind="ExternalInput")
    outp = nc.dram_tensor("outp", [TOK, C], BF16, kind="ExternalOutput")

    xT_r = xT[:, :].rearrange("(a p) t -> p a t", p=128)  # [128, KC, TOK]

    with tile.TileContext(nc) as tc:
        with (
            tc.tile_pool(name="const", bufs=1) as cpool,
            tc.tile_pool(name="xt", bufs=4) as xpool,
            tc.tile_pool(name="qk", bufs=2) as qkpool,
            tc.tile_pool(name="vv", bufs=2) as vpool,
            tc.tile_pool(name="yy", bufs=2) as ypool,
            tc.tile_pool(name="small", bufs=2) as spool,
            tc.tile_pool(name="den", bufs=2) as dpool,
            tc.tile_pool(name="pt", bufs=4) as ptpool,
            tc.tile_pool(name="outs", bufs=4) as opool,
            tc.tile_pool(name="dram", bufs=2, space="DRAM") as drampool,
            tc.tile_pool(name="ps_mm", bufs=2, space="PSUM") as ps_mm,
            tc.tile_pool(name="ps_s", bufs=2, space="PSUM") as ps_s,
            tc.tile_pool(name="ps_y", bufs=1, space="PSUM") as ps_y,
        ):
            # ---- constants (priority order: first QKV chunk's deps first) ----
            w_sb = {}
            b_sb = {}

            def load_w(name, dram, dt=BF16):
                t = cpool.tile([128, KC, FW], dt, tag=name)
                for g in range(0, KC, KC // 2):
                    nc.sync.dma_start(
                        t[:, g : g + KC // 2, :],
                        dram[:, :].rearrange("(a p) f -> p a f", p=128)[
                            :, g : g + KC // 2, :
                        ],
                    )
                w_sb[name] = t

            def load_b(bname, bias_d):
                bt = cpool.tile([FW, 1], F32, tag=bname)
                nc.sync.dma_start(bt[:, :], bias_d[:, :])
                b_sb[bname] = bt

            perm_sb = cpool.tile([128, 128], BF16, tag="perm")
            cos_sb = cpool.tile([FW, T], BF16, tag="cos")
            sin_sb = cpool.tile([FW, T], BF16, tag="sin")
            mask_sb = cpool.tile([128, 4 * 512], BF16, tag="mask")
            wp2_sb = cpool.tile([FW, C], BF16, tag="wp2")

            def late_consts():
                for g in range(0, T, T // 2):
                    nc.sync.dma_start(
                        cos_sb[:, g : g + T // 2], cosT[:, g : g + T // 2]
                    )
                    nc.sync.dma_start(
                        sin_sb[:, g : g + T // 2], sinT[:, g : g + T // 2]
                    )
                nc.sync.dma_start(mask_sb[:, :], mask4[:, :])
                nc.sync.dma_start(wp2_sb[:, :], wp[:, :])

            ident = cpool.tile([128, 128], BF16, tag="ident")
            from concourse.masks import make_identity

            make_identity(nc, ident[:, :])
            ones_raw = cpool.tile([128, 128], F32, tag="ones_raw")
            nc.vector.memset(ones_raw[:, :], 1.0)
            # warm the PE p-state while the first input DMAs are in flight
            warm = ps_mm.tile([128, 128], F32, tag="mm", name="warm")
            for wi in range(40):
                nc.tensor.matmul(warm[:, :], lhsT=ident[:, :], rhs=ident[:, :],
                                 start=(wi == 0), stop=(wi == 39),
                                 skip_group_check=True)

            state = {}
            # PE/DVE ops deferred two units so PE never waits on ACT evacs
            defq = deque()  # (due_unit, b, cn, fn)
            unit_ctr = [0]

            def defer(b, cn, fn):
                defq.append((unit_ctr[0] + 2, b, cn, fn))

            def run_deferred():
                while defq and defq[0][0] <= unit_ctr[0]:
                    defq.popleft()[3]()

            def flush_deferred(b, ic):
                while defq and (defq[0][1] < b or
                                (defq[0][1] == b and defq[0][2] <= ic)):
                    defq.popleft()[3]()

            def alloc_qkv(b):
                st = state.setdefault(b, {})
                st["qT"] = qkpool.tile([FW, T], BF16, tag="qT", name=f"qT{b}")
                # per-head K tiles, other head's rows zeroed: keeps the S
                # stationary a full 128x128 tile (no PE tile-config switch)
                st["kT0"] = qkpool.tile([FW, T], BF16, tag="kT0", name=f"kT0{b}")
                st["kT1"] = qkpool.tile([FW, T], BF16, tag="kT1", name=f"kT1{b}")
                nc.vector.memset(st["kT0"][HD:FW, :], 0.0)
                nc.vector.memset(st["kT1"][0:HD, :], 0.0)
                # v combined: per j-tile and head, [v_h (64) | ones | zeros(63)]
                # so the AV stationary is also a full 128x128 tile
                st["vc"] = vpool.tile([128, NJT, 256], BF16, tag="vc",
                                      name=f"vc{b}")
                for h in range(2):
                    nc.vector.tensor_copy(
                        st["vc"][:, :, h * 128 + HD],
                        ones_raw[:, 0:1].broadcast_to([128, NJT]),
                    )
                    nc.vector.memset(
                        st["vc"][:, :, h * 128 + HD + 1 : h * 128 + 128], 0.0
                    )

            def fetch_xt(b, fcn):
                st = state.setdefault(b, {})
                if ("xt", fcn) in st or fcn >= NCH:
                    return
                fx = xpool.tile([128, KC, 512], BF16, tag="xt",
                                name=f"xt{b}_{fcn}")
                st[("xt", fcn)] = fx
                fs0 = b * T + fcn * 512
                for g in range(0, KC, 2):
                    nc.sync.dma_start(
                        fx[:, g : g + 2, :],
                        xT_r[:, g : g + 2, fs0 : fs0 + 512],
                    )

            def unit_qkv(b, cn, name):
                def fn():
                    unit_ctr[0] += 1
                    run_deferred()
                    if "qT" not in state.setdefault(b, {}):
                        alloc_qkv(b)
                    st = state[b]
                    tok0 = b * T
                    ts0 = cn * 512
                    if name == "wq":
                        fetch_xt(b, cn)
                        fetch_xt(b, cn + 1)
                    xt = st[("xt", cn)]
                    ps = ps_mm.tile([128, 512], F32, tag="mm",
                                    name=f"qkvps{b}_{cn}_{name}")
                    for kc in range(KC):
                        nc.tensor.matmul(
                            ps[:, :],
                            lhsT=w_sb[name][:, kc, :],
                            rhs=xt[:, kc, :],
                            start=(kc == 0),
                            stop=(kc == KC - 1),
                        )
                    if name == "wv":
                        st.pop(("xt", cn))
                        vch = spool.tile([128, 512], BF16, tag="vch",
                                         name=f"vch{b}_{cn}")
                        nc.scalar.activation(
                            vch[:, :], ps[:, :], AF.Identity, bias=b_sb["bv"][:, :]
                        )

                        def dtrans():
                            vc = state[b]["vc"]
                            for qd in range(4):
                                pst = ps_mm.tile([128, 128], BF16, tag="mm",
                                                 name=f"pst{b}_{cn}_{qd}")
                                nc.tensor.transpose(
                                    pst[:, :],
                                    vch[:, qd * 128 : qd * 128 + 128],
                                    ident[:, :],
                                )
                                jt = cn * 4 + qd
                                nc.vector.tensor_copy(
                                    vc[:, jt, :].rearrange(
                                        "p (a c) -> p a c", a=2
                                    )[:, :, 0:HD],
                                    pst[:, :].rearrange("p (a c) -> p a c", a=2),
                                )

                        defer(b, cn, dtrans)
                    else:
                        if name == "wq":
                            dch = st["qT"][:, ts0 : ts0 + 512]
                            bias = b_sb["bq"]
                        else:
                            ktmp = spool.tile([128, 512], BF16, tag="ktmp",
                                              name=f"ktmp{b}_{cn}")
                            dch = ktmp[:, :]
                            bias = b_sb["bk"]
                        nc.scalar.activation(dch, ps[:, :], AF.Identity, bias=bias[:, :])

                        def drope():
                            swp = ps_mm.tile([128, 512], F32, tag="mm",
                                             name=f"swp{b}_{cn}_{name}")
                            nc.tensor.matmul(
                                swp[:, :], lhsT=perm_sb[:, :], rhs=dch,
                                start=True, stop=True,
                            )
                            cc = cos_sb[:, ts0 : ts0 + 512]
                            ss = sin_sb[:, ts0 : ts0 + 512]
                            t1 = spool.tile([128, 512], F32, tag="t1",
                                            name=f"t1{b}_{cn}_{name}")
                            t2 = spool.tile([128, 512], F32, tag="t2",
                                            name=f"t2{b}_{cn}_{name}")
                            eng = nc.vector if name == "wq" else nc.gpsimd
                            eng.tensor_tensor(t1[:, :], dch, cc, op=OP.mult)
                            nc.vector.tensor_tensor(t2[:, :], swp[:, :], ss,
                                                    op=OP.mult)
                            if name == "wq":
                                eng.tensor_tensor(dch, t1[:, :], t2[:, :],
                                                  op=OP.add)
                            else:
                                eng.tensor_tensor(
                                    st["kT0"][0:HD, ts0 : ts0 + 512],
                                    t1[0:HD, :], t2[0:HD, :], op=OP.add,
                                )
                                eng.tensor_tensor(
                                    st["kT1"][HD:FW, ts0 : ts0 + 512],
                                    t1[HD:FW, :], t2[HD:FW, :], op=OP.add,
                                )

                        defer(b, cn, drope)

                return fn

            # ---- filler machinery ----
            pending = deque()   # (b, cn, name, fn) in emission order
            proj_q = deque()    # (b, fn)
            tail_stash = []

            def pop_filler(cur_b):
                if pending and pending[0][0] <= cur_b + 1:
                    pending.popleft()[3]()
                elif proj_q:
                    proj_q.popleft()[1]()

            def force_qkv(b, cn):
                while pending and (
                    pending[0][0] < b
                    or (pending[0][0] == b and pending[0][1] <= cn)
                ):
                    pending.popleft()[3]()

            def drain_proj(max_b):
                while proj_q and proj_q[0][0] <= max_b:
                    proj_q.popleft()[1]()

            # ---- attention ----
            def attn_begin(b, ic):
                st = state[b]
                yps = {
                    h: ps_y.tile([128, 512], F32, tag=f"y{h}",
                                 name=f"yps{b}_{ic}_{h}")
                    for h in range(2)
                }
                st[("yps", ic)] = yps
                if ic == 0:
                    st["yTfull"] = ypool.tile([FW, T], BF16, tag="yTfull",
                                              name=f"yTfull{b}")
                    # den rows: per-half 16-row blocks at 32-aligned partitions
                    st["den_all"] = dpool.tile([32 + 8 * HALF, 128], F32,
                                               tag="den", name=f"den{b}")
                    st["rec_all"] = dpool.tile([32 + 8 * HALF, 128], F32,
                                               tag="rec", name=f"rec{b}")
                    st["rec_bf"] = dpool.tile([32 + 8 * HALF, 128], BF16,
                                              tag="recbf", name=f"recbf{b}")
                    st["rec_dram"] = drampool.tile([2 * NCH, 512], BF16,
                                                   tag="rec_dram",
                                                   name=f"rec_dram{b}")

            def emit_S(b, ic, jg):
                st = state[b]
                i0 = ic * 512
                njt = 4 * (ic + 1)
                dv0 = 2 * jg - (njt - 4)
                scale = float(1.0 / np.sqrt(HD))
                pt = {}
                for h in range(2):
                    kz = st["kT0"] if h == 0 else st["kT1"]
                    sp = ps_s.tile([128, 1024], F32, tag="s",
                                   name=f"sps{b}_{ic}_{jg}_{h}")
                    for li in range(2):
                        jt = 2 * jg + li
                        off = max(jt - (njt - 4), 0) * 128
                        nc.tensor.matmul(
                            sp[:, li * 512 + off : li * 512 + 512],
                            lhsT=kz[:, jt * 128 : jt * 128 + 128],
                            rhs=st["qT"][:, i0 + off : i0 + 512],
                            start=True,
                            stop=True,
                        )
                    p = ptpool.tile([128, 1024], BF16, tag="pt",
                                    name=f"pt{b}_{ic}_{jg}_{h}")
                    if dv0 < 0:
                        nc.scalar.activation(p[:, :], sp[:, :], AF.Exp,
                                             scale=scale)
                    else:
                        for li in range(2):
                            dv = dv0 + li
                            off = li * 512 + dv * 128
                            nc.scalar.activation(
                                p[:, off : li * 512 + 512],
                                sp[:, off : li * 512 + 512],
                                AF.Exp, scale=scale,
                            )
                    pt[h] = p
                st[("pt", ic, jg)] = pt

            def emit_exp_mask(b, ic, jg):
                st = state[b]
                njt = 4 * (ic + 1)
                dv0 = 2 * jg - (njt - 4)
                pt = st[("pt", ic, jg)]
                if dv0 >= 0:  # mask only the 128-wide diagonal square per tile
                    for h in range(2):
                        for li in range(2):
                            dv = dv0 + li
                            po = li * 512 + dv * 128
                            mo = dv * 512 + dv * 128
                            nc.vector.tensor_tensor(
                                pt[h][:, po : po + 128],
                                pt[h][:, po : po + 128],
                                mask_sb[:, mo : mo + 128],
                                op=OP.mult,
                            )

            def emit_AV(b, ic, jg):
                st = state[b]
                njt = 4 * (ic + 1)
                pt = st.pop(("pt", ic, jg))
                yps = st[("yps", ic)]
                for h in range(2):
                    for li in range(2):
                        jt = 2 * jg + li
                        off = max(jt - (njt - 4), 0) * 128
                        nc.tensor.matmul(
                            yps[h][:, off:512],
                            lhsT=st["vc"][:, jt, h * 128 : h * 128 + 128],
                            rhs=pt[h][:, li * 512 + off : li * 512 + 512],
                            start=(jt == 0),
                            stop=(jt == njt - 1),
                            skip_group_check=True,
                        )

            def attn_end(b, ic):
                st = state[b]
                yps = st.pop(("yps", ic))
                i0 = ic * 512
                yTfull = st["yTfull"]
                dent = spool.tile([HD + 1, 1024], F32, tag="dent",
                                  name=f"dent{b}_{ic}")
                base = (ic // HALF) * 32 + (ic % HALF) * 8
                for h in range(2):
                    nc.vector.tensor_copy(
                        dent[HD : HD + 1, h * 512 : h * 512 + 512],
                        yps[h][HD : HD + 1, :],
                    )
                    b4 = base + h * 4
                    nc.sync.dma_start(
                        st["den_all"][b4 : b4 + 4, :],
                        dent[HD : HD + 1, h * 512 : h * 512 + 512],
                    )
                nc.vector.tensor_copy(yTfull[0:HD, i0 : i0 + 512], yps[0][0:HD, :])
                ytmp = spool.tile([HD, 512], BF16, tag="ytmp", name=f"ytmp{b}_{ic}")
                nc.vector.tensor_copy(ytmp[:, :], yps[1][0:HD, :])
                nc.sync.dma_start(yTfull[HD:FW, i0 : i0 + 512], ytmp[:, :])

            def emit_recip_half(b, half):
                st = state[b]
                base = half * 32
                n = 8 * HALF
                nc.vector.reciprocal(
                    st["rec_all"][base : base + n, :],
                    st["den_all"][base : base + n, :],
                )
                nc.vector.tensor_copy(
                    st["rec_bf"][base : base + n, :],
                    st["rec_all"][base : base + n, :],
                )
                nc.sync.dma_start(
                    st["rec_dram"][half * NCH : half * NCH + 2 * HALF, :],
                    st["rec_bf"][base : base + n, :],
                )
                for ic in range(half * HALF, (half + 1) * HALF):
                    ro = ic * 2
                    rb = spool.tile([128, 512], BF16, tag="rb",
                                    name=f"rb{b}_{ic}")
                    nc.sync.dma_start(
                        rb[0:HD, :],
                        st["rec_dram"][ro : ro + 1, :].broadcast_to([HD, 512]),
                    )
                    nc.sync.dma_start(
                        rb[HD:128, :],
                        st["rec_dram"][ro + 1 : ro + 2, :].broadcast_to(
                            [HD, 512]
                        ),
                    )
                    st[("rb", ic)] = rb

            def unit_proj(b, ic, half):
                def fn():
                    unit_ctr[0] += 1
                    run_deferred()
                    st = state[b]
                    tok0 = b * T
                    i0 = ic * 512
                    yTfull = st["yTfull"]
                    if half == 0 and ("rb", ic) in st:
                        rb = st.pop(("rb", ic))
                        nc.vector.tensor_tensor(
                            yTfull[:, i0 : i0 + 512], yTfull[:, i0 : i0 + 512],
                            rb[:, :], op=OP.mult,
                        )
                    for tt in ((0, 1) if half == 0 else (2, 3)):
                        tr0 = i0 + tt * 128
                        for fc in range(C // 512):
                            pp = ps_mm.tile([128, 512], F32, tag="mm",
                                            name=f"pp{b}_{ic}_{tt}_{fc}")
                            nc.tensor.matmul(
                                pp[:, :],
                                lhsT=yTfull[:, tr0 : tr0 + 128],
                                rhs=wp2_sb[:, fc * 512 : fc * 512 + 512],
                                start=True,
                                stop=True,
                            )
                            ot = opool.tile([128, 512], BF16, tag="ot",
                                            name=f"ot{b}_{ic}_{tt}_{fc}")
                            if (tt * 2 + fc) % 2 == 0:
                                nc.vector.tensor_copy(ot[:, :], pp[:, :])
                            else:
                                nc.scalar.activation(ot[:, :], pp[:, :], AF.Copy)
                            nc.sync.dma_start(
                                outp[tok0 + tr0 : tok0 + tr0 + 128,
                                     fc * 512 : fc * 512 + 512],
                                ot[:, :],
                            )

                return fn

            # ---- schedule ----
            fetch_xt(0, 0)
            load_w("wq", wq)
            load_b("bq", bq)
            load_w("wk", wk)
            load_b("bk", bk)
            fetch_xt(0, 1)
            load_w("wv", wv)
            load_b("bv", bv)
            nc.sync.dma_start(perm_sb[:, :], perm[:, :])
            late_consts()

            for b in range(B):
                for cn in range(NCH):
                    for name in ("wq", "wk", "wv"):
                        pending.append((b, cn, name, unit_qkv(b, cn, name)))

            for b in range(B):
                for ic in range(NCH):
                    force_qkv(b, ic)
                    flush_deferred(b, ic)
                    if ic == 0:
                        drain_proj(b - 2)
                    attn_begin(b, ic)
                    njg = 2 * (ic + 1)
                    prev = None
                    for jg in range(njg):
                        emit_S(b, ic, jg)
                        emit_exp_mask(b, ic, jg)
                        pop_filler(b)
                        if prev is not None:
                            emit_AV(b, ic, prev)
                        prev = jg
                    pop_filler(b)
                    emit_AV(b, ic, prev)
                    attn_end(b, ic)
                    if ic % HALF == HALF - 1:
                        half = ic // HALF
                        emit_recip_half(b, half)
                        for pic in range(half * HALF, (half + 1) * HALF):
                            u0 = (b, unit_proj(b, pic, 0))
                            u1 = (b, unit_proj(b, pic, 1))
                            if b == B - 1 and half == 0 and pic == HALF - 1:
                                tail_stash.extend([u0, u1])
                            else:
                                proj_q.append(u0)
                                proj_q.append(u1)
                    if b == B - 1 and ic == NCH - 1:
                        proj_q.extendleft(reversed(tail_stash))
                        tail_stash.clear()
            flush_deferred(B, NCH)
            while pending:
                pending.popleft()[3]()
            while proj_q:
                proj_q.popleft()[1]()
    if split:
        _split_waits(nc)
    return nc


# ---------------------------------------------------------------- host side


def make_tables(T):
    inv_freq = 1.0 / (ROPE_BASE ** (np.arange(0, HD, 2, dtype=np.float32) / HD))
    pos = np.arange(T, dtype=np.float32)
    freqs = pos[:, None] * inv_freq[None, :]  # [T, 32]
    cos = np.cos(freqs).astype(np.float32)  # [T, 32] (same for both halves)
    sin = np.sin(freqs).astype(np.float32)
    cosT64 = np.concatenate([cos.T, cos.T], axis=0)  # [64, T]
    sinT64 = np.concatenate([-sin.T, sin.T], axis=0)  # sign-baked rotate_half
    cosT = np.concatenate([cosT64, cosT64], axis=0).copy()  # [128, T] two heads
    sinT = np.concatenate([sinT64, sinT64], axis=0).copy()
    return cosT, sinT


def make_perm():
    # perm[k, m] = 1 iff m == (k+32) % 64 within each 64-row head block
    p = np.zeros((128, 128), dtype=np.float32)
    for hb in range(2):
        for k in range(HD):
            p[hb * HD + k, hb * HD + (k + 32) % HD] = 1.0
    return p


def make_mask4():
    # mask4[p, v*512 + f] = 1.0 if v*128 + p <= f else 0.0
    m = np.zeros((128, 4 * 512), dtype=np.float32)
    p = np.arange(128)[:, None]
    f = np.arange(512)[None, :]
    for v in range(4):
        m[:, v * 512 : (v + 1) * 512] = (v * 128 + p <= f).astype(np.float32)
    return m


def make_in_maps(x, W_qkv, b_qkv, W_proj, n_cores):
    B, T, C = x.shape
    import ml_dtypes

    xT = np.ascontiguousarray(x.reshape(B * T, C).T.astype(ml_dtypes.bfloat16))
    cosT, sinT = make_tables(T)
    mask4 = make_mask4()
    perm = make_perm()
    in_maps = []
    for c in range(n_cores):
        h0 = 2 * c * HD  # first head's column offset (2 heads per core)
        sl = slice(h0, h0 + 128)
        in_maps.append(
            {
                "xT": xT,
                "wq": np.ascontiguousarray(W_qkv[:, sl].astype(ml_dtypes.bfloat16)),
                "wk": np.ascontiguousarray(
                    W_qkv[:, C:][:, sl].astype(ml_dtypes.bfloat16)
                ),
                "wv": np.ascontiguousarray(
                    W_qkv[:, 2 * C :][:, sl].astype(ml_dtypes.bfloat16)
                ),
                "bq": np.ascontiguousarray(b_qkv[sl].reshape(128, 1)),
                "bk": np.ascontiguousarray(b_qkv[C:][sl].reshape(128, 1)),
                "bv": np.ascontiguousarray(b_qkv[2 * C :][sl].reshape(128, 1)),
                "wp": np.ascontiguousarray(W_proj[sl, :].astype(ml_dtypes.bfloat16)),
                "cosT": np.ascontiguousarray(cosT.astype(ml_dtypes.bfloat16)),
                "sinT": np.ascontiguousarray(sinT.astype(ml_dtypes.bfloat16)),
                "perm": perm.astype(ml_dtypes.bfloat16),
                "mask4": mask4.astype(ml_dtypes.bfloat16),
            }
        )
    return in_maps


_NC_CACHE = {}


def _get_nc(B, T, C):
    key = (B, T, C)
    if key not in _NC_CACHE:
        _NC_CACHE[key] = build_nc(B, T, C)
    return _NC_CACHE[key]


def kernel(x, W_qkv, b_qkv, W_proj, b_proj):
    from concourse.bass_utils import run_bass_kernel_spmd

    x = np.asarray(x, dtype=np.float32)
    W_qkv = np.asarray(W_qkv, dtype=np.float32)
    b_qkv = np.asarray(b_qkv, dtype=np.float32)
    W_proj = np.asarray(W_proj, dtype=np.float32)
    b_proj = np.asarray(b_proj, dtype=np.float32)
    B, T, C = x.shape
    n_cores = 8
    nc = _get_nc(B, T, C)
    in_maps = make_in_maps(x, W_qkv, b_qkv, W_proj, n_cores)
    res = run_bass_kernel_spmd(nc, in_maps, core_ids=list(range(n_cores)))
    out = np.zeros((B * T, C), dtype=np.float32)
    for r in res.results:
        out += r["outp"].astype(np.float32)
    out += b_proj[None, :]
    return out.reshape(B, T, C)


# revision 29
# speedup vs baseline: 1.0033x; 1.0033x over previous
"""Causal self-attention (RoPE) Trainium2 kernel, 8-way head-parallel.

Sharding: each of the 8 cores computes 2 of the 16 heads for all 4 batches
(tensor parallel over heads: W_qkv column-split, W_proj row-split). Host
pre-transposes x -> xT [C, B*T], slices per-core weights, and sum-reduces the
8 partial projection outputs (+ b_proj) — the standard row-parallel TP reduce.

Per-core dataflow (bf16 storage/matmuls, fp32 PSUM):
  qkvT = W_slice.T @ xT            [feat, tok] PSUM, bias added on ACT evac
  RoPE on qT,kT                    (rotate-half via permutation matmul on PE)
  v: PE-transpose vT -> vc tiles   [tok, 2*(HD+1)] (+ ones col per head)
  per (b, h, i-chunk, jg of 2 j-tiles):
      S^T = kT_jt.T @ qT_ichunk    (j on partitions)
      P^T = exp(S^T/8) (ACT), causal mask on diagonal tiles (mult, DVE)
      [yT_h | denom] += vc_h.T @ P^T   accumulated over j-tiles in PSUM
  yT_h *= recip(denom) (approx recip, bf16 bcast), out = sum_h yT_h.T @ Wp_h

The emission schedule software-pipelines at j-group granularity: between a
j-group's S matmuls and the PREVIOUS group's AV matmuls we pop one "filler"
unit (a QKV weight-stream for batch b+1 or a projection half for an earlier
chunk) so the PE stream stays dense while ACT computes exp.
"""

from collections import deque

import numpy as np

import concourse.bass as bass
import concourse.mybir as mybir
import concourse.tile as tile

F32 = mybir.dt.float32
BF16 = mybir.dt.bfloat16
AF = mybir.ActivationFunctionType
OP = mybir.AluOpType

# ---------------------------------------------------------------- tile patch
# This walrus build rejects >1 embedded sync-wait on sync-engine CTRL
# instructions; Tile's tail drain embeds one wait per outstanding semaphore.
# Split them across NOPs (1 wait each) before the drain.


def _patched_drain_and_barrier(self, tick_clock, wait_clock):
    from concourse.tile import ScopedClock

    nc = self.nc
    probe = nc.sync.nop(nofuse=True)
    wait_clock.add_sem_waits(probe.ins, ScopedClock({None: tick_clock.global_clock}))
    si = probe.ins.sync_info
    waits = list(si.on_wait) if si is not None and si.on_wait else []
    if len(waits) > 1:
        si.on_wait = waits[:1]
        for w in waits[1:]:
            nop = nc.sync.nop(nofuse=True)
            nsi = nop.ins.sync_info
            if nsi is None:
                nop.ins.sync_info = mybir.SyncInfo(on_wait=[w], on_update=[])
            else:
                nsi.on_wait = [w]
    nc.sync.drain()
    nc.all_engine_barrier()
    assert self.sems is not None
    popped = nc._tile_sem_poison_stack.pop()
    assert popped is self._sem_poison
    # chunk the sem clears: the range-encoded gpsimd drain (dma_reset) in this
    # walrus build rejects wide semaphore ranges ("ISA wrong length")
    sems = sorted(
        s.num if hasattr(s, "num") else s for s in self.sems.allocated().values()
    )
    for i in range(0, len(sems), 16):
        nc.clear_and_free_semaphores(sems[i : i + 16])
    nc.all_engine_barrier()


tile.TileContext._drain_and_barrier = _patched_drain_and_barrier


def _split_waits(nc):
    """Hoist all but one sync-wait per instruction onto same-engine NOPs
    (this walrus codegen supports a single embedded wait per instruction)."""
    n = 0
    for f in nc.m.functions:
        for blk in f.blocks:
            out = []
            changed = False
            for ins in blk.instructions:
                si = ins.sync_info
                if si is not None and si.on_wait and len(si.on_wait) > 1:
                    waits = list(si.on_wait)
                    for w in waits[:-1]:
                        n += 1
                        out.append(
                            mybir.InstNoOp(
                                name=f"wsplit{n}",
                                engine=ins.engine,
                                sync_info=mybir.SyncInfo(on_wait=[w], on_update=[]),
                                bass_nofuse=True,
                            )
                        )
                    si.on_wait = waits[-1:]
                    changed = True
                out.append(ins)
            if changed:
                blk.instructions = out
    return n


# ------------------------------------------------------------------- builder

HD = 64  # head dim (fixed)
ROPE_BASE = 10000.0


def build_nc(B, T, C, split=True):
    """One core's program: 2 heads x B batches. T % 512 == 0, C % 128 == 0."""
    assert T % 512 == 0 and C % 128 == 0
    TOK = B * T
    KC = C // 128   # contraction chunks for QKV
    NCH = T // 512  # i-chunks per batch
    NJT = T // 128  # j-tiles per batch
    FW = 128        # qkv feature width per tensor (2 heads * 64)
    HALF = NCH // 2

    nc = bass.Bass()
    xT = nc.dram_tensor("xT", [C, TOK], BF16, kind="ExternalInput")
    wq = nc.dram_tensor("wq", [C, FW], BF16, kind="ExternalInput")
    wk = nc.dram_tensor("wk", [C, FW], BF16, kind="ExternalInput")
    wv = nc.dram_tensor("wv", [C, FW], BF16, kind="ExternalInput")
    bq = nc.dram_tensor("bq", [FW, 1], F32, kind="ExternalInput")
    bk = nc.dram_tensor("bk", [FW, 1], F32, kind="ExternalInput")
    bv = nc.dram_tensor("bv", [FW, 1], F32, kind="ExternalInput")
    wp = nc.dram_tensor("wp", [FW, C], BF16, kind="ExternalInput")
    cosT = nc.dram_tensor("cosT", [FW, T], BF16, kind="ExternalInput")
    sinT = nc.dram_tensor("sinT", [FW, T], BF16, kind="ExternalInput")
    perm = nc.dram_tensor("perm", [128, 128], BF16, kind="ExternalInput")
    mask4 = nc.dram_tensor("mask4", [128, 4 * 512], BF16, kind="ExternalInput")
    outp = nc.dram_tensor("outp", [TOK, C], BF16, kind="ExternalOutput")

    xT_r = xT[:, :].rearrange("(a p) t -> p a t", p=128)  # [128, KC, TOK]

    with tile.TileContext(nc) as tc:
        with (
            tc.tile_pool(name="const", bufs=1) as cpool,
            tc.tile_pool(name="xt", bufs=4) as xpool,
            tc.tile_pool(name="qk", bufs=2) as qkpool,
            tc.tile_pool(name="vv", bufs=2) as vpool,
            tc.tile_pool(name="yy", bufs=2) as ypool,
            tc.tile_pool(name="small", bufs=2) as spool,
            tc.tile_pool(name="den", bufs=2) as dpool,
            tc.tile_pool(name="pt", bufs=4) as ptpool,
            tc.tile_pool(name="outs", bufs=4) as opool,
            tc.tile_pool(name="dram", bufs=2, space="DRAM") as drampool,
            tc.tile_pool(name="ps_mm", bufs=2, space="PSUM") as ps_mm,
            tc.tile_pool(name="ps_s", bufs=2, space="PSUM") as ps_s,
            tc.tile_pool(name="ps_y", bufs=1, space="PSUM") as ps_y,
        ):
            # ---- constants (priority order: first QKV chunk's deps first) ----
            w_sb = {}
            b_sb = {}

            def load_w(name, dram, dt=BF16):
                t = cpool.tile([128, KC, FW], dt, tag=name)
                for g in range(0, KC, KC // 2):
                    nc.sync.dma_start(
                        t[:, g : g + KC // 2, :],
                        dram[:, :].rearrange("(a p) f -> p a f", p=128)[
                            :, g : g + KC // 2, :
                        ],
                    )
                w_sb[name] = t

            def load_b(bname, bias_d):
                bt = cpool.tile([FW, 1], F32, tag=bname)
                nc.sync.dma_start(bt[:, :], bias_d[:, :])
                b_sb[bname] = bt

            perm_sb = cpool.tile([128, 128], BF16, tag="perm")
            cos_sb = cpool.tile([FW, T], BF16, tag="cos")
            sin_sb = cpool.tile([FW, T], BF16, tag="sin")
            mask_sb = cpool.tile([128, 4 * 512], BF16, tag="mask")
            wp2_sb = cpool.tile([FW, C], BF16, tag="wp2")

            def late_consts():
                for g in range(0, T, T // 2):
                    nc.sync.dma_start(
                        cos_sb[:, g : g + T // 2], cosT[:, g : g + T // 2]
                    )
                    nc.sync.dma_start(
                        sin_sb[:, g : g + T // 2], sinT[:, g : g + T // 2]
                    )
                nc.sync.dma_start(mask_sb[:, :], mask4[:, :])
                nc.sync.dma_start(wp2_sb[:, :], wp[:, :])

            ident = cpool.tile([128, 128], BF16, tag="ident")
            from concourse.masks import make_identity

            make_identity(nc, ident[:, :])
            ones_raw = cpool.tile([128, 128], F32, tag="ones_raw")
            nc.vector.memset(ones_raw[:, :], 1.0)

            state = {}
            # PE/DVE ops deferred two units so PE never waits on ACT evacs
            defq = deque()  # (due_unit, b, cn, fn)
            unit_ctr = [0]

            def defer(b, cn, fn):
                defq.append((unit_ctr[0] + 2, b, cn, fn))

            def run_deferred():
                while defq and defq[0][0] <= unit_ctr[0]:
                    defq.popleft()[3]()

            def flush_deferred(b, ic):
                while defq and (defq[0][1] < b or
                                (defq[0][1] == b and defq[0][2] <= ic)):
                    defq.popleft()[3]()

            def alloc_qkv(b):
                st = state.setdefault(b, {})
                st["qT"] = qkpool.tile([FW, T], BF16, tag="qT", name=f"qT{b}")
                # per-head K tiles, other head's rows zeroed: keeps the S
                # stationary a full 128x128 tile (no PE tile-config switch)
                st["kT0"] = qkpool.tile([FW, T], BF16, tag="kT0", name=f"kT0{b}")
                st["kT1"] = qkpool.tile([FW, T], BF16, tag="kT1", name=f"kT1{b}")
                nc.vector.memset(st["kT0"][HD:FW, :], 0.0)
                nc.vector.memset(st["kT1"][0:HD, :], 0.0)
                # v combined: per j-tile and head, [v_h (64) | ones | zeros(63)]
                # so the AV stationary is also a full 128x128 tile
                st["vc"] = vpool.tile([128, NJT, 256], BF16, tag="vc",
                                      name=f"vc{b}")
                for h in range(2):
                    nc.vector.tensor_copy(
                        st["vc"][:, :, h * 128 + HD],
                        ones_raw[:, 0:1].broadcast_to([128, NJT]),
                    )
                    nc.vector.memset(
                        st["vc"][:, :, h * 128 + HD + 1 : h * 128 + 128], 0.0
                    )

            def fetch_xt(b, fcn):
                st = state.setdefault(b, {})
                if ("xt", fcn) in st or fcn >= NCH:
                    return
                fx = xpool.tile([128, KC, 512], BF16, tag="xt",
                                name=f"xt{b}_{fcn}")
                st[("xt", fcn)] = fx
                fs0 = b * T + fcn * 512
                for g in range(0, KC, 2):
                    nc.sync.dma_start(
                        fx[:, g : g + 2, :],
                        xT_r[:, g : g + 2, fs0 : fs0 + 512],
                    )

            def unit_qkv(b, cn, name):
                def fn():
                    unit_ctr[0] += 1
                    run_deferred()
                    if "qT" not in state.setdefault(b, {}):
                        alloc_qkv(b)
                    st = state[b]
                    tok0 = b * T
                    ts0 = cn * 512
                    if name == "wq":
                        fetch_xt(b, cn)
                        fetch_xt(b, cn + 1)
                    xt = st[("xt", cn)]
                    ps = ps_mm.tile([128, 512], F32, tag="mm",
                                    name=f"qkvps{b}_{cn}_{name}")
                    for kc in range(KC):
                        nc.tensor.matmul(
                            ps[:, :],
                            lhsT=w_sb[name][:, kc, :],
                            rhs=xt[:, kc, :],
                            start=(kc == 0),
                            stop=(kc == KC - 1),
                        )
                    if name == "wv":
                        st.pop(("xt", cn))
                        vch = spool.tile([128, 512], BF16, tag="vch",
                                         name=f"vch{b}_{cn}")
                        nc.scalar.activation(
                            vch[:, :], ps[:, :], AF.Identity, bias=b_sb["bv"][:, :]
                        )

                        def dtrans():
                            vc = state[b]["vc"]
                            for qd in range(4):
                                pst = ps_mm.tile([128, 128], BF16, tag="mm",
                                                 name=f"pst{b}_{cn}_{qd}")
                                nc.tensor.transpose(
                                    pst[:, :],
                                    vch[:, qd * 128 : qd * 128 + 128],
                                    ident[:, :],
                                )
                                jt = cn * 4 + qd
                                nc.vector.tensor_copy(
                                    vc[:, jt, :].rearrange(
                                        "p (a c) -> p a c", a=2
                                    )[:, :, 0:HD],
                                    pst[:, :].rearrange("p (a c) -> p a c", a=2),
                                )

                        defer(b, cn, dtrans)
                    else:
                        if name == "wq":
                            dch = st["qT"][:, ts0 : ts0 + 512]
                            bias = b_sb["bq"]
                        else:
                            ktmp = spool.tile([128, 512], BF16, tag="ktmp",
                                              name=f"ktmp{b}_{cn}")
                            dch = ktmp[:, :]
                            bias = b_sb["bk"]
                        nc.scalar.activation(dch, ps[:, :], AF.Identity, bias=bias[:, :])

                        def drope():
                            swp = ps_mm.tile([128, 512], F32, tag="mm",
                                             name=f"swp{b}_{cn}_{name}")
                            nc.tensor.matmul(
                                swp[:, :], lhsT=perm_sb[:, :], rhs=dch,
                                start=True, stop=True,
                            )
                            cc = cos_sb[:, ts0 : ts0 + 512]
                            ss = sin_sb[:, ts0 : ts0 + 512]
                            t1 = spool.tile([128, 512], F32, tag="t1",
                                            name=f"t1{b}_{cn}_{name}")
                            t2 = spool.tile([128, 512], F32, tag="t2",
                                            name=f"t2{b}_{cn}_{name}")
                            eng = nc.vector if name == "wq" else nc.gpsimd
                            eng.tensor_tensor(t1[:, :], dch, cc, op=OP.mult)
                            nc.vector.tensor_tensor(t2[:, :], swp[:, :], ss,
                                                    op=OP.mult)
                            if name == "wq":
                                eng.tensor_tensor(dch, t1[:, :], t2[:, :],
                                                  op=OP.add)
                            else:
                                eng.tensor_tensor(
                                    st["kT0"][0:HD, ts0 : ts0 + 512],
                                    t1[0:HD, :], t2[0:HD, :], op=OP.add,
                                )
                                eng.tensor_tensor(
                                    st["kT1"][HD:FW, ts0 : ts0 + 512],
                                    t1[HD:FW, :], t2[HD:FW, :], op=OP.add,
                                )

                        defer(b, cn, drope)

                return fn

            # ---- filler machinery ----
            pending = deque()   # (b, cn, name, fn) in emission order
            proj_q = deque()    # (b, fn)
            tail_stash = []

            def pop_filler(cur_b):
                if pending and pending[0][0] <= cur_b + 1:
                    pending.popleft()[3]()
                elif proj_q:
                    proj_q.popleft()[1]()

            def force_qkv(b, cn):
                while pending and (
                    pending[0][0] < b
                    or (pending[0][0] == b and pending[0][1] <= cn)
                ):
                    pending.popleft()[3]()

            def drain_proj(max_b):
                while proj_q and proj_q[0][0] <= max_b:
                    proj_q.popleft()[1]()

            # ---- attention ----
            def attn_begin(b, ic):
                st = state[b]
                yps = {
                    h: ps_y.tile([128, 512], F32, tag=f"y{h}",
                                 name=f"yps{b}_{ic}_{h}")
                    for h in range(2)
                }
                st[("yps", ic)] = yps
                if ic == 0:
                    st["yTfull"] = ypool.tile([FW, T], BF16, tag="yTfull",
                                              name=f"yTfull{b}")
                    # den rows: per-ic 8-row blocks at 32-aligned partitions
                    st["den_all"] = dpool.tile([32 * (NCH - 1) + 8, 128], F32,
                                               tag="den", name=f"den{b}")
                    st["rec_all"] = dpool.tile([32 * (NCH - 1) + 8, 128], F32,
                                               tag="rec", name=f"rec{b}")
                    st["rec_bf"] = dpool.tile([32 * (NCH - 1) + 8, 128], BF16,
                                              tag="recbf", name=f"recbf{b}")
                    st["rec_dram"] = drampool.tile([2 * NCH, 512], BF16,
                                                   tag="rec_dram",
                                                   name=f"rec_dram{b}")

            def emit_S(b, ic, jg):
                st = state[b]
                i0 = ic * 512
                njt = 4 * (ic + 1)
                dv0 = 2 * jg - (njt - 4)
                scale = float(1.0 / np.sqrt(HD))
                pt = {}
                for h in range(2):
                    kz = st["kT0"] if h == 0 else st["kT1"]
                    sp = ps_s.tile([128, 1024], F32, tag="s",
                                   name=f"sps{b}_{ic}_{jg}_{h}")
                    for li in range(2):
                        jt = 2 * jg + li
                        off = max(jt - (njt - 4), 0) * 128
                        nc.tensor.matmul(
                            sp[:, li * 512 + off : li * 512 + 512],
                            lhsT=kz[:, jt * 128 : jt * 128 + 128],
                            rhs=st["qT"][:, i0 + off : i0 + 512],
                            start=True,
                            stop=True,
                        )
                    p = ptpool.tile([128, 1024], BF16, tag="pt",
                                    name=f"pt{b}_{ic}_{jg}_{h}")
                    if dv0 < 0:
                        nc.scalar.activation(p[:, :], sp[:, :], AF.Exp,
                                             scale=scale)
                    else:
                        for li in range(2):
                            dv = dv0 + li
                            off = li * 512 + dv * 128
                            nc.scalar.activation(
                                p[:, off : li * 512 + 512],
                                sp[:, off : li * 512 + 512],
                                AF.Exp, scale=scale,
                            )
                    pt[h] = p
                st[("pt", ic, jg)] = pt

            def emit_exp_mask(b, ic, jg):
                st = state[b]
                njt = 4 * (ic + 1)
                dv0 = 2 * jg - (njt - 4)
                pt = st[("pt", ic, jg)]
                if dv0 >= 0:  # mask only the 128-wide diagonal square per tile
                    for h in range(2):
                        for li in range(2):
                            dv = dv0 + li
                            po = li * 512 + dv * 128
                            mo = dv * 512 + dv * 128
                            nc.vector.tensor_tensor(
                                pt[h][:, po : po + 128],
                                pt[h][:, po : po + 128],
                                mask_sb[:, mo : mo + 128],
                                op=OP.mult,
                            )

            def emit_AV(b, ic, jg):
                st = state[b]
                njt = 4 * (ic + 1)
                pt = st.pop(("pt", ic, jg))
                yps = st[("yps", ic)]
                for h in range(2):
                    for li in range(2):
                        jt = 2 * jg + li
                        off = max(jt - (njt - 4), 0) * 128
                        nc.tensor.matmul(
                            yps[h][:, off:512],
                            lhsT=st["vc"][:, jt, h * 128 : h * 128 + 128],
                            rhs=pt[h][:, li * 512 + off : li * 512 + 512],
                            start=(jt == 0),
                            stop=(jt == njt - 1),
                            skip_group_check=True,
                        )

            def attn_end(b, ic):
                st = state[b]
                yps = st.pop(("yps", ic))
                i0 = ic * 512
                yTfull = st["yTfull"]
                if b == B - 1 and ic == NCH - 1 and ("rb", ic - 1) in st:
                    prb = st.pop(("rb", ic - 1))
                    pi0 = (ic - 1) * 512
                    nc.vector.tensor_tensor(
                        yTfull[:, pi0 : pi0 + 512], yTfull[:, pi0 : pi0 + 512],
                        prb[:, :], op=OP.mult,
                    )
                dent = spool.tile([HD + 1, 1024], F32, tag="dent",
                                  name=f"dent{b}_{ic}")
                base = ic * 32
                for h in range(2):
                    nc.vector.tensor_copy(
                        dent[HD : HD + 1, h * 512 : h * 512 + 512],
                        yps[h][HD : HD + 1, :],
                    )
                    b4 = base + h * 4
                    nc.sync.dma_start(
                        st["den_all"][b4 : b4 + 4, :],
                        dent[HD : HD + 1, h * 512 : h * 512 + 512],
                    )
                nc.vector.reciprocal(
                    st["rec_all"][base : base + 8, :],
                    st["den_all"][base : base + 8, :],
                )
                nc.vector.tensor_copy(
                    st["rec_bf"][base : base + 8, :],
                    st["rec_all"][base : base + 8, :],
                )
                ro = ic * 2
                nc.sync.dma_start(
                    st["rec_dram"][ro : ro + 2, :],
                    st["rec_bf"][base : base + 8, :],
                )
                rb = spool.tile([128, 512], BF16, tag="rb", name=f"rb{b}_{ic}")
                nc.sync.dma_start(
                    rb[0:HD, :],
                    st["rec_dram"][ro : ro + 1, :].broadcast_to([HD, 512]),
                )
                nc.sync.dma_start(
                    rb[HD:128, :],
                    st["rec_dram"][ro + 1 : ro + 2, :].broadcast_to([HD, 512]),
                )
                nc.vector.tensor_copy(yTfull[0:HD, i0 : i0 + 512], yps[0][0:HD, :])
                ytmp = spool.tile([HD, 512], BF16, tag="ytmp", name=f"ytmp{b}_{ic}")
                nc.vector.tensor_copy(ytmp[:, :], yps[1][0:HD, :])
                nc.sync.dma_start(yTfull[HD:FW, i0 : i0 + 512], ytmp[:, :])
                st[("rb", ic)] = rb

            def unit_proj(b, ic, half):
                def fn():
                    unit_ctr[0] += 1
                    run_deferred()
                    st = state[b]
                    tok0 = b * T
                    i0 = ic * 512
                    yTfull = st["yTfull"]
                    if half == 0 and ("rb", ic) in st:
                        rb = st.pop(("rb", ic))
                        nc.vector.tensor_tensor(
                            yTfull[:, i0 : i0 + 512], yTfull[:, i0 : i0 + 512],
                            rb[:, :], op=OP.mult,
                        )
                    for tt in ((0, 1) if half == 0 else (2, 3)):
                        tr0 = i0 + tt * 128
                        for fc in range(C // 512):
                            pp = ps_mm.tile([128, 512], F32, tag="mm",
                                            name=f"pp{b}_{ic}_{tt}_{fc}")
                            nc.tensor.matmul(
                                pp[:, :],
                                lhsT=yTfull[:, tr0 : tr0 + 128],
                                rhs=wp2_sb[:, fc * 512 : fc * 512 + 512],
                                start=True,
                                stop=True,
                            )
                            ot = opool.tile([128, 512], BF16, tag="ot",
                                            name=f"ot{b}_{ic}_{tt}_{fc}")
                            if (tt * 2 + fc) % 2 == 0:
                                nc.vector.tensor_copy(ot[:, :], pp[:, :])
                            else:
                                nc.scalar.activation(ot[:, :], pp[:, :], AF.Copy)
                            nc.sync.dma_start(
                                outp[tok0 + tr0 : tok0 + tr0 + 128,
                                     fc * 512 : fc * 512 + 512],
                                ot[:, :],
                            )

                return fn

            # ---- schedule ----
            fetch_xt(0, 0)
            load_w("wq", wq)
            load_b("bq", bq)
            load_w("wk", wk)
            load_b("bk", bk)
            fetch_xt(0, 1)
            load_w("wv", wv)
            load_b("bv", bv)
            nc.sync.dma_start(perm_sb[:, :], perm[:, :])
            late_consts()

            for b in range(B):
                for cn in range(NCH):
                    for name in ("wq", "wk", "wv"):
                        pending.append((b, cn, name, unit_qkv(b, cn, name)))

            for b in range(B):
                for ic in range(NCH):
                    force_qkv(b, ic)
                    flush_deferred(b, ic)
                    if ic == 0:
                        drain_proj(b - 2)
                    attn_begin(b, ic)
                    njg = 2 * (ic + 1)
                    prev = None
                    for jg in range(njg):
                        emit_S(b, ic, jg)
                        emit_exp_mask(b, ic, jg)
                        pop_filler(b)
                        if prev is not None:
                            emit_AV(b, ic, prev)
                        prev = jg
                    pop_filler(b)
                    emit_AV(b, ic, prev)
                    attn_end(b, ic)
                    if b == B - 1 and ic == NCH - 2:
                        tail_stash.extend(
                            [(b, unit_proj(b, ic, 0)), (b, unit_proj(b, ic, 1))]
                        )
                    else:
                        proj_q.append((b, unit_proj(b, ic, 0)))
                        proj_q.append((b, unit_proj(b, ic, 1)))
                    if b == B - 1 and ic == NCH - 1:
                        proj_q.extendleft(reversed(tail_stash))
                        tail_stash.clear()
            flush_deferred(B, NCH)
            while pending:
                pending.popleft()[3]()
            while proj_q:
                proj_q.popleft()[1]()
    if split:
        _split_waits(nc)
    return nc


# ---------------------------------------------------------------- host side


def make_tables(T):
    inv_freq = 1.0 / (ROPE_BASE ** (np.arange(0, HD, 2, dtype=np.float32) / HD))
    pos = np.arange(T, dtype=np.float32)
    freqs = pos[:, None] * inv_freq[None, :]  # [T, 32]
    cos = np.cos(freqs).astype(np.float32)  # [T, 32] (same for both halves)
    sin = np.sin(freqs).astype(np.float32)
    cosT64 = np.concatenate([cos.T, cos.T], axis=0)  # [64, T]
    sinT64 = np.concatenate([-sin.T, sin.T], axis=0)  # sign-baked rotate_half
    cosT = np.concatenate([cosT64, cosT64], axis=0).copy()  # [128, T] two heads
    sinT = np.concatenate([sinT64, sinT64], axis=0).copy()
    return cosT, sinT


def make_perm():
    # perm[k, m] = 1 iff m == (k+32) % 64 within each 64-row head block
    p = np.zeros((128, 128), dtype=np.float32)
    for hb in range(2):
        for k in range(HD):
            p[hb * HD + k, hb * HD + (k + 32) % HD] = 1.0
    return p


def make_mask4():
    # mask4[p, v*512 + f] = 1.0 if v*128 + p <= f else 0.0
    m = np.zeros((128, 4 * 512), dtype=np.float32)
    p = np.arange(128)[:, None]
    f = np.arange(512)[None, :]
    for v in range(4):
        m[:, v * 512 : (v + 1) * 512] = (v * 128 + p <= f).astype(np.float32)
    return m


def make_in_maps(x, W_qkv, b_qkv, W_proj, n_cores):
    B, T, C = x.shape
    import ml_dtypes

    xT = np.ascontiguousarray(x.reshape(B * T, C).T.astype(ml_dtypes.bfloat16))
    cosT, sinT = make_tables(T)
    mask4 = make_mask4()
    perm = make_perm()
    in_maps = []
    for c in range(n_cores):
        h0 = 2 * c * HD  # first head's column offset (2 heads per core)
        sl = slice(h0, h0 + 128)
        in_maps.append(
            {
                "xT": xT,
                "wq": np.ascontiguousarray(W_qkv[:, sl].astype(ml_dtypes.bfloat16)),
                "wk": np.ascontiguousarray(
                    W_qkv[:, C:][:, sl].astype(ml_dtypes.bfloat16)
                ),
                "wv": np.ascontiguousarray(
                    W_qkv[:, 2 * C :][:, sl].astype(ml_dtypes.bfloat16)
                ),
                "bq": np.ascontiguousarray(b_qkv[sl].reshape(128, 1)),
                "bk": np.ascontiguousarray(b_qkv[C:][sl].reshape(128, 1)),
                "bv": np.ascontiguousarray(b_qkv[2 * C :][sl].reshape(128, 1)),
                "wp": np.ascontiguousarray(W_proj[sl, :].astype(ml_dtypes.bfloat16)),
                "cosT": np.ascontiguousarray(cosT.astype(ml_dtypes.bfloat16)),
                "sinT": np.ascontiguousarray(sinT.astype(ml_dtypes.bfloat16)),
                "perm": perm.astype(ml_dtypes.bfloat16),
                "mask4": mask4.astype(ml_dtypes.bfloat16),
            }
        )
    return in_maps


_NC_CACHE = {}


def _get_nc(B, T, C):
    key = (B, T, C)
    if key not in _NC_CACHE:
        _NC_CACHE[key] = build_nc(B, T, C)
    return _NC_CACHE[key]


def kernel(x, W_qkv, b_qkv, W_proj, b_proj):
    from concourse.bass_utils import run_bass_kernel_spmd

    x = np.asarray(x, dtype=np.float32)
    W_qkv = np.asarray(W_qkv, dtype=np.float32)
    b_qkv = np.asarray(b_qkv, dtype=np.float32)
    W_proj = np.asarray(W_proj, dtype=np.float32)
    b_proj = np.asarray(b_proj, dtype=np.float32)
    B, T, C = x.shape
    n_cores = 8
    nc = _get_nc(B, T, C)
    in_maps = make_in_maps(x, W_qkv, b_qkv, W_proj, n_cores)
    res = run_bass_kernel_spmd(nc, in_maps, core_ids=list(range(n_cores)))
    out = np.zeros((B * T, C), dtype=np.float32)
    for r in res.results:
        out += r["outp"].astype(np.float32)
    out += b_proj[None, :]
    return out.reshape(B, T, C)


# revision 30
# speedup vs baseline: 1.0246x; 1.0212x over previous
"""Causal self-attention (RoPE) Trainium2 kernel, 8-way head-parallel.

Sharding: each of the 8 cores computes 2 of the 16 heads for all 4 batches
(tensor parallel over heads: W_qkv column-split, W_proj row-split). Host
pre-transposes x -> xT [C, B*T], slices per-core weights, and sum-reduces the
8 partial projection outputs (+ b_proj) — the standard row-parallel TP reduce.

Per-core dataflow (bf16 storage/matmuls, fp32 PSUM):
  qkvT = W_slice.T @ xT            [feat, tok] PSUM, bias added on ACT evac
  RoPE on qT,kT                    (rotate-half via permutation matmul on PE)
  v: PE-transpose vT -> vc tiles   [tok, 2*(HD+1)] (+ ones col per head)
  per (b, h, i-chunk, jg of 2 j-tiles):
      S^T = kT_jt.T @ qT_ichunk    (j on partitions)
      P^T = exp(S^T/8) (ACT), causal mask on diagonal tiles (mult, DVE)
      [yT_h | denom] += vc_h.T @ P^T   accumulated over j-tiles in PSUM
  yT_h *= recip(denom) (approx recip, bf16 bcast), out = sum_h yT_h.T @ Wp_h

The emission schedule software-pipelines at j-group granularity: between a
j-group's S matmuls and the PREVIOUS group's AV matmuls we pop one "filler"
unit (a QKV weight-stream for batch b+1 or a projection half for an earlier
chunk) so the PE stream stays dense while ACT computes exp.
"""

from collections import deque

import numpy as np

import concourse.bass as bass
import concourse.mybir as mybir
import concourse.tile as tile

F32 = mybir.dt.float32
BF16 = mybir.dt.bfloat16
AF = mybir.ActivationFunctionType
OP = mybir.AluOpType

# ---------------------------------------------------------------- tile patch
# This walrus build rejects >1 embedded sync-wait on sync-engine CTRL
# instructions; Tile's tail drain embeds one wait per outstanding semaphore.
# Split them across NOPs (1 wait each) before the drain.


def _patched_drain_and_barrier(self, tick_clock, wait_clock):
    from concourse.tile import ScopedClock

    nc = self.nc
    probe = nc.sync.nop(nofuse=True)
    wait_clock.add_sem_waits(probe.ins, ScopedClock({None: tick_clock.global_clock}))
    si = probe.ins.sync_info
    waits = list(si.on_wait) if si is not None and si.on_wait else []
    if len(waits) > 1:
        si.on_wait = waits[:1]
        for w in waits[1:]:
            nop = nc.sync.nop(nofuse=True)
            nsi = nop.ins.sync_info
            if nsi is None:
                nop.ins.sync_info = mybir.SyncInfo(on_wait=[w], on_update=[])
            else:
                nsi.on_wait = [w]
    nc.sync.drain()
    nc.all_engine_barrier()
    assert self.sems is not None
    popped = nc._tile_sem_poison_stack.pop()
    assert popped is self._sem_poison
    # chunk the sem clears: the range-encoded gpsimd drain (dma_reset) in this
    # walrus build rejects wide semaphore ranges ("ISA wrong length")
    sems = sorted(
        s.num if hasattr(s, "num") else s for s in self.sems.allocated().values()
    )
    for i in range(0, len(sems), 16):
        nc.clear_and_free_semaphores(sems[i : i + 16])
    nc.all_engine_barrier()


tile.TileContext._drain_and_barrier = _patched_drain_and_barrier


def _split_waits(nc):
    """Hoist all but one sync-wait per instruction onto same-engine NOPs
    (this walrus codegen supports a single embedded wait per instruction)."""
    n = 0
    for f in nc.m.functions:
        for blk in f.blocks:
            out = []
            changed = False
            for ins in blk.instructions:
                si = ins.sync_info
                if si is not None and si.on_wait and len(si.on_wait) > 1:
                    waits = list(si.on_wait)
                    for w in waits[:-1]:
                        n += 1
                        out.append(
                            mybir.InstNoOp(
                                name=f"wsplit{n}",
                                engine=ins.engine,
                                sync_info=mybir.SyncInfo(on_wait=[w], on_update=[]),
                                bass_nofuse=True,
                            )
                        )
                    si.on_wait = waits[-1:]
                    changed = True
                out.append(ins)
            if changed:
                blk.instructions = out
    return n


# ------------------------------------------------------------------- builder

HD = 64  # head dim (fixed)
ROPE_BASE = 10000.0


def build_nc(B, T, C, split=True):
    """One core's program: 2 heads x B batches. T % 512 == 0, C % 128 == 0."""
    assert T % 512 == 0 and C % 128 == 0
    TOK = B * T
    KC = C // 128   # contraction chunks for QKV
    NCH = T // 512  # i-chunks per batch
    NJT = T // 128  # j-tiles per batch
    FW = 128        # qkv feature width per tensor (2 heads * 64)
    HALF = NCH // 2

    nc = bass.Bass()
    xT = nc.dram_tensor("xT", [C, TOK], BF16, kind="ExternalInput")
    wq = nc.dram_tensor("wq", [C, FW], BF16, kind="ExternalInput")
    wk = nc.dram_tensor("wk", [C, FW], BF16, kind="ExternalInput")
    wv = nc.dram_tensor("wv", [C, FW], BF16, kind="ExternalInput")
    bq = nc.dram_tensor("bq", [FW, 1], F32, kind="ExternalInput")
    bk = nc.dram_tensor("bk", [FW, 1], F32, kind="ExternalInput")
    bv = nc.dram_tensor("bv", [FW, 1], F32, kind="ExternalInput")
    wp = nc.dram_tensor("wp", [FW, C], BF16, kind="ExternalInput")
    cosT = nc.dram_tensor("cosT", [FW, T], BF16, kind="ExternalInput")
    sinT = nc.dram_tensor("sinT", [FW, T], BF16, kind="ExternalInput")
    perm = nc.dram_tensor("perm", [128, 128], BF16, kind="ExternalInput")
    mask4 = nc.dram_tensor("mask4", [128, 4 * 512], BF16, kind="ExternalInput")
    outp = nc.dram_tensor("outp", [TOK, C], BF16, kind="ExternalOutput")

    xT_r = xT[:, :].rearrange("(a p) t -> p a t", p=128)  # [128, KC, TOK]

    with tile.TileContext(nc) as tc:
        with (
            tc.tile_pool(name="const", bufs=1) as cpool,
            tc.tile_pool(name="xt", bufs=4) as xpool,
            tc.tile_pool(name="qk", bufs=2) as qkpool,
            tc.tile_pool(name="vv", bufs=2) as vpool,
            tc.tile_pool(name="yy", bufs=2) as ypool,
            tc.tile_pool(name="small", bufs=2) as spool,
            tc.tile_pool(name="den", bufs=2) as dpool,
            tc.tile_pool(name="pt", bufs=4) as ptpool,
            tc.tile_pool(name="outs", bufs=4) as opool,
            tc.tile_pool(name="dram", bufs=2, space="DRAM") as drampool,
            tc.tile_pool(name="ps_mm", bufs=2, space="PSUM") as ps_mm,
            tc.tile_pool(name="ps_s", bufs=2, space="PSUM") as ps_s,
            tc.tile_pool(name="ps_y", bufs=1, space="PSUM") as ps_y,
        ):
            # ---- constants (priority order: first QKV chunk's deps first) ----
            w_sb = {}
            b_sb = {}

            def load_w(name, dram, dt=BF16):
                t = cpool.tile([128, KC, FW], dt, tag=name)
                for g in range(0, KC, KC // 2):
                    nc.sync.dma_start(
                        t[:, g : g + KC // 2, :],
                        dram[:, :].rearrange("(a p) f -> p a f", p=128)[
                            :, g : g + KC // 2, :
                        ],
                    )
                w_sb[name] = t

            def load_b(bname, bias_d):
                bt = cpool.tile([FW, 1], F32, tag=bname)
                nc.sync.dma_start(bt[:, :], bias_d[:, :])
                b_sb[bname] = bt

            perm_sb = cpool.tile([128, 128], BF16, tag="perm")
            cos_sb = cpool.tile([FW, T], BF16, tag="cos")
            sin_sb = cpool.tile([FW, T], BF16, tag="sin")
            mask_sb = cpool.tile([128, 4 * 512], BF16, tag="mask")
            wp2_sb = cpool.tile([FW, C], BF16, tag="wp2")

            def late_consts():
                for g in range(0, T, T // 2):
                    nc.sync.dma_start(
                        cos_sb[:, g : g + T // 2], cosT[:, g : g + T // 2]
                    )
                    nc.sync.dma_start(
                        sin_sb[:, g : g + T // 2], sinT[:, g : g + T // 2]
                    )
                nc.sync.dma_start(mask_sb[:, :], mask4[:, :])
                nc.sync.dma_start(wp2_sb[:, :], wp[:, :])

            ident = cpool.tile([128, 128], BF16, tag="ident")
            from concourse.masks import make_identity

            make_identity(nc, ident[:, :])
            ones_raw = cpool.tile([128, 128], F32, tag="ones_raw")
            nc.vector.memset(ones_raw[:, :], 1.0)

            state = {}
            # PE/DVE ops deferred two units so PE never waits on ACT evacs
            defq = deque()  # (due_unit, b, cn, fn)
            unit_ctr = [0]

            def defer(b, cn, fn):
                defq.append((unit_ctr[0] + 2, b, cn, fn))

            def run_deferred():
                while defq and defq[0][0] <= unit_ctr[0]:
                    defq.popleft()[3]()

            def flush_deferred(b, ic):
                while defq and (defq[0][1] < b or
                                (defq[0][1] == b and defq[0][2] <= ic)):
                    defq.popleft()[3]()

            def alloc_qkv(b):
                st = state.setdefault(b, {})
                st["qT"] = qkpool.tile([FW, T], BF16, tag="qT", name=f"qT{b}")
                # per-head K tiles, other head's rows zeroed: keeps the S
                # stationary a full 128x128 tile (no PE tile-config switch)
                st["kT0"] = qkpool.tile([FW, T], BF16, tag="kT0", name=f"kT0{b}")
                st["kT1"] = qkpool.tile([FW, T], BF16, tag="kT1", name=f"kT1{b}")
                nc.vector.memset(st["kT0"][HD:FW, :], 0.0)
                nc.vector.memset(st["kT1"][0:HD, :], 0.0)
                # v combined: per j-tile and head, [v_h (64) | ones | zeros(63)]
                # so the AV stationary is also a full 128x128 tile
                st["vc"] = vpool.tile([128, NJT, 256], BF16, tag="vc",
                                      name=f"vc{b}")
                for h in range(2):
                    nc.vector.tensor_copy(
                        st["vc"][:, :, h * 128 + HD],
                        ones_raw[:, 0:1].broadcast_to([128, NJT]),
                    )
                    nc.vector.memset(
                        st["vc"][:, :, h * 128 + HD + 1 : h * 128 + 128], 0.0
                    )

            def fetch_xt(b, fcn):
                st = state.setdefault(b, {})
                if ("xt", fcn) in st or fcn >= NCH:
                    return
                fx = xpool.tile([128, KC, 512], BF16, tag="xt",
                                name=f"xt{b}_{fcn}")
                st[("xt", fcn)] = fx
                fs0 = b * T + fcn * 512
                for g in range(0, KC, 2):
                    nc.sync.dma_start(
                        fx[:, g : g + 2, :],
                        xT_r[:, g : g + 2, fs0 : fs0 + 512],
                    )

            def unit_qkv(b, cn, name):
                def fn():
                    unit_ctr[0] += 1
                    run_deferred()
                    if "qT" not in state.setdefault(b, {}):
                        alloc_qkv(b)
                    st = state[b]
                    tok0 = b * T
                    ts0 = cn * 512
                    if name == "wq":
                        fetch_xt(b, cn)
                        fetch_xt(b, cn + 1)
                    xt = st[("xt", cn)]
                    ps = ps_mm.tile([128, 512], F32, tag="mm",
                                    name=f"qkvps{b}_{cn}_{name}")
                    for kc in range(KC):
                        nc.tensor.matmul(
                            ps[:, :],
                            lhsT=w_sb[name][:, kc, :],
                            rhs=xt[:, kc, :],
                            start=(kc == 0),
                            stop=(kc == KC - 1),
                        )
                    if name == "wv":
                        st.pop(("xt", cn))
                        vch = spool.tile([128, 512], BF16, tag="vch",
                                         name=f"vch{b}_{cn}")
                        nc.scalar.activation(
                            vch[:, :], ps[:, :], AF.Identity, bias=b_sb["bv"][:, :]
                        )

                        def dtrans():
                            vc = state[b]["vc"]
                            for qd in range(4):
                                pst = ps_mm.tile([128, 128], BF16, tag="mm",
                                                 name=f"pst{b}_{cn}_{qd}")
                                nc.tensor.transpose(
                                    pst[:, :],
                                    vch[:, qd * 128 : qd * 128 + 128],
                                    ident[:, :],
                                )
                                jt = cn * 4 + qd
                                nc.vector.tensor_copy(
                                    vc[:, jt, :].rearrange(
                                        "p (a c) -> p a c", a=2
                                    )[:, :, 0:HD],
                                    pst[:, :].rearrange("p (a c) -> p a c", a=2),
                                )

                        defer(b, cn, dtrans)
                    else:
                        if name == "wq":
                            dch = st["qT"][:, ts0 : ts0 + 512]
                            bias = b_sb["bq"]
                        else:
                            ktmp = spool.tile([128, 512], BF16, tag="ktmp",
                                              name=f"ktmp{b}_{cn}")
                            dch = ktmp[:, :]
                            bias = b_sb["bk"]
                        nc.scalar.activation(dch, ps[:, :], AF.Identity, bias=bias[:, :])

                        def drope():
                            swp = ps_mm.tile([128, 512], F32, tag="mm",
                                             name=f"swp{b}_{cn}_{name}")
                            nc.tensor.matmul(
                                swp[:, :], lhsT=perm_sb[:, :], rhs=dch,
                                start=True, stop=True,
                            )
                            cc = cos_sb[:, ts0 : ts0 + 512]
                            ss = sin_sb[:, ts0 : ts0 + 512]
                            t1 = spool.tile([128, 512], F32, tag="t1",
                                            name=f"t1{b}_{cn}_{name}")
                            t2 = spool.tile([128, 512], F32, tag="t2",
                                            name=f"t2{b}_{cn}_{name}")
                            eng = nc.vector if name == "wq" else nc.gpsimd
                            eng.tensor_tensor(t1[:, :], dch, cc, op=OP.mult)
                            nc.vector.tensor_tensor(t2[:, :], swp[:, :], ss,
                                                    op=OP.mult)
                            if name == "wq":
                                eng.tensor_tensor(dch, t1[:, :], t2[:, :],
                                                  op=OP.add)
                            else:
                                eng.tensor_tensor(
                                    st["kT0"][0:HD, ts0 : ts0 + 512],
                                    t1[0:HD, :], t2[0:HD, :], op=OP.add,
                                )
                                eng.tensor_tensor(
                                    st["kT1"][HD:FW, ts0 : ts0 + 512],
                                    t1[HD:FW, :], t2[HD:FW, :], op=OP.add,
                                )

                        defer(b, cn, drope)

                return fn

            # ---- filler machinery ----
            pending = deque()   # (b, cn, name, fn) in emission order
            proj_q = deque()    # (b, fn)
            tail_stash = []

            def pop_filler(cur_b):
                if pending and pending[0][0] <= cur_b + 1:
                    pending.popleft()[3]()
                elif proj_q:
                    proj_q.popleft()[1]()

            def force_qkv(b, cn):
                while pending and (
                    pending[0][0] < b
                    or (pending[0][0] == b and pending[0][1] <= cn)
                ):
                    pending.popleft()[3]()

            def drain_proj(max_b):
                while proj_q and proj_q[0][0] <= max_b:
                    proj_q.popleft()[1]()

            # ---- attention ----
            def attn_begin(b, ic):
                st = state[b]
                yps = {
                    h: ps_y.tile([128, 512], F32, tag=f"y{h}",
                                 name=f"yps{b}_{ic}_{h}")
                    for h in range(2)
                }
                st[("yps", ic)] = yps
                if ic == 0:
                    st["yTfull"] = ypool.tile([FW, T], BF16, tag="yTfull",
                                              name=f"yTfull{b}")
                    # den rows: per-ic 8-row blocks at 32-aligned partitions
                    st["den_all"] = dpool.tile([32 * (NCH - 1) + 8, 128], F32,
                                               tag="den", name=f"den{b}")
                    st["rec_all"] = dpool.tile([32 * (NCH - 1) + 8, 128], F32,
                                               tag="rec", name=f"rec{b}")
                    st["rec_bf"] = dpool.tile([32 * (NCH - 1) + 8, 128], BF16,
                                              tag="recbf", name=f"recbf{b}")
                    st["rec_dram"] = drampool.tile([2 * NCH, 512], BF16,
                                                   tag="rec_dram",
                                                   name=f"rec_dram{b}")

            def emit_S(b, ic, jg):
                st = state[b]
                i0 = ic * 512
                njt = 4 * (ic + 1)
                dv0 = 2 * jg - (njt - 4)
                scale = float(1.0 / np.sqrt(HD))
                pt = {}
                for h in range(2):
                    kz = st["kT0"] if h == 0 else st["kT1"]
                    sp = ps_s.tile([128, 1024], F32, tag="s",
                                   name=f"sps{b}_{ic}_{jg}_{h}")
                    for li in range(2):
                        jt = 2 * jg + li
                        off = max(jt - (njt - 4), 0) * 128
                        nc.tensor.matmul(
                            sp[:, li * 512 + off : li * 512 + 512],
                            lhsT=kz[:, jt * 128 : jt * 128 + 128],
                            rhs=st["qT"][:, i0 + off : i0 + 512],
                            start=True,
                            stop=True,
                        )
                    p = ptpool.tile([128, 1024], BF16, tag="pt",
                                    name=f"pt{b}_{ic}_{jg}_{h}")
                    if dv0 < 0:
                        nc.scalar.activation(p[:, :], sp[:, :], AF.Exp,
                                             scale=scale)
                    else:
                        for li in range(2):
                            dv = dv0 + li
                            off = li * 512 + dv * 128
                            nc.scalar.activation(
                                p[:, off : li * 512 + 512],
                                sp[:, off : li * 512 + 512],
                                AF.Exp, scale=scale,
                            )
                    pt[h] = p
                st[("pt", ic, jg)] = pt

            def emit_exp_mask(b, ic, jg):
                st = state[b]
                njt = 4 * (ic + 1)
                dv0 = 2 * jg - (njt - 4)
                pt = st[("pt", ic, jg)]
                if dv0 >= 0:  # mask only the 128-wide diagonal square per tile
                    for h in range(2):
                        for li in range(2):
                            dv = dv0 + li
                            po = li * 512 + dv * 128
                            mo = dv * 512 + dv * 128
                            nc.vector.tensor_tensor(
                                pt[h][:, po : po + 128],
                                pt[h][:, po : po + 128],
                                mask_sb[:, mo : mo + 128],
                                op=OP.mult,
                            )

            def emit_AV(b, ic, jg):
                st = state[b]
                njt = 4 * (ic + 1)
                pt = st.pop(("pt", ic, jg))
                yps = st[("yps", ic)]
                for h in range(2):
                    for li in range(2):
                        jt = 2 * jg + li
                        off = max(jt - (njt - 4), 0) * 128
                        nc.tensor.matmul(
                            yps[h][:, off:512],
                            lhsT=st["vc"][:, jt, h * 128 : h * 128 + 128],
                            rhs=pt[h][:, li * 512 + off : li * 512 + 512],
                            start=(jt == 0),
                            stop=(jt == njt - 1),
                            skip_group_check=True,
                        )

            def attn_end(b, ic):
                st = state[b]
                yps = st.pop(("yps", ic))
                i0 = ic * 512
                yTfull = st["yTfull"]
                if b == B - 1 and ic == NCH - 1:
                    for pic in (ic - 2, ic - 1):
                        if ("rb", pic) in st:
                            prb = st.pop(("rb", pic))
                            pi0 = pic * 512
                            nc.vector.tensor_tensor(
                                yTfull[:, pi0 : pi0 + 512],
                                yTfull[:, pi0 : pi0 + 512],
                                prb[:, :], op=OP.mult,
                            )
                dent = spool.tile([HD + 1, 1024], F32, tag="dent",
                                  name=f"dent{b}_{ic}")
                base = ic * 32
                for h in range(2):
                    nc.vector.tensor_copy(
                        dent[HD : HD + 1, h * 512 : h * 512 + 512],
                        yps[h][HD : HD + 1, :],
                    )
                    b4 = base + h * 4
                    nc.sync.dma_start(
                        st["den_all"][b4 : b4 + 4, :],
                        dent[HD : HD + 1, h * 512 : h * 512 + 512],
                    )
                nc.vector.reciprocal(
                    st["rec_all"][base : base + 8, :],
                    st["den_all"][base : base + 8, :],
                )
                nc.vector.tensor_copy(
                    st["rec_bf"][base : base + 8, :],
                    st["rec_all"][base : base + 8, :],
                )
                ro = ic * 2
                nc.sync.dma_start(
                    st["rec_dram"][ro : ro + 2, :],
                    st["rec_bf"][base : base + 8, :],
                )
                rb = spool.tile([128, 512], BF16, tag="rb", name=f"rb{b}_{ic}")
                nc.sync.dma_start(
                    rb[0:HD, :],
                    st["rec_dram"][ro : ro + 1, :].broadcast_to([HD, 512]),
                )
                nc.sync.dma_start(
                    rb[HD:128, :],
                    st["rec_dram"][ro + 1 : ro + 2, :].broadcast_to([HD, 512]),
                )
                nc.vector.tensor_copy(yTfull[0:HD, i0 : i0 + 512], yps[0][0:HD, :])
                ytmp = spool.tile([HD, 512], BF16, tag="ytmp", name=f"ytmp{b}_{ic}")
                nc.vector.tensor_copy(ytmp[:, :], yps[1][0:HD, :])
                nc.sync.dma_start(yTfull[HD:FW, i0 : i0 + 512], ytmp[:, :])
                st[("rb", ic)] = rb

            def unit_proj(b, ic, tt):
                def fn():
                    run_deferred()
                    st = state[b]
                    tok0 = b * T
                    i0 = ic * 512
                    yTfull = st["yTfull"]
                    if tt == 0 and ("rb", ic) in st:
                        rb = st.pop(("rb", ic))
                        nc.vector.tensor_tensor(
                            yTfull[:, i0 : i0 + 512], yTfull[:, i0 : i0 + 512],
                            rb[:, :], op=OP.mult,
                        )
                    if True:
                        tr0 = i0 + tt * 128
                        for fc in range(C // 512):
                            pp = ps_mm.tile([128, 512], F32, tag="mm",
                                            name=f"pp{b}_{ic}_{tt}_{fc}")
                            nc.tensor.matmul(
                                pp[:, :],
                                lhsT=yTfull[:, tr0 : tr0 + 128],
                                rhs=wp2_sb[:, fc * 512 : fc * 512 + 512],
                                start=True,
                                stop=True,
                            )
                            ot = opool.tile([128, 512], BF16, tag="ot",
                                            name=f"ot{b}_{ic}_{tt}_{fc}")
                            if (tt * 2 + fc) % 2 == 0:
                                nc.vector.tensor_copy(ot[:, :], pp[:, :])
                            else:
                                nc.scalar.activation(ot[:, :], pp[:, :], AF.Copy)
                            nc.sync.dma_start(
                                outp[tok0 + tr0 : tok0 + tr0 + 128,
                                     fc * 512 : fc * 512 + 512],
                                ot[:, :],
                            )

                return fn

            # ---- schedule ----
            fetch_xt(0, 0)
            load_w("wq", wq)
            load_b("bq", bq)
            load_w("wk", wk)
            load_b("bk", bk)
            fetch_xt(0, 1)
            load_w("wv", wv)
            load_b("bv", bv)
            nc.sync.dma_start(perm_sb[:, :], perm[:, :])
            late_consts()

            for b in range(B):
                for cn in range(NCH):
                    for name in ("wq", "wk", "wv"):
                        pending.append((b, cn, name, unit_qkv(b, cn, name)))

            for b in range(B):
                for ic in range(NCH):
                    force_qkv(b, ic)
                    flush_deferred(b, ic)
                    if ic == 0:
                        drain_proj(b - 2)
                    attn_begin(b, ic)
                    njg = 2 * (ic + 1)
                    prev = None
                    for jg in range(njg):
                        emit_S(b, ic, jg)
                        emit_exp_mask(b, ic, jg)
                        pop_filler(b)
                        if prev is not None:
                            emit_AV(b, ic, prev)
                        prev = jg
                    pop_filler(b)
                    emit_AV(b, ic, prev)
                    attn_end(b, ic)
                    units = [(b, unit_proj(b, ic, tt)) for tt in range(4)]
                    if b == B - 1 and ic in (NCH - 3, NCH - 2):
                        tail_stash.extend(units)
                    else:
                        proj_q.extend(units)
                    if b == B - 1 and ic == NCH - 1:
                        proj_q.extendleft(reversed(tail_stash))
                        tail_stash.clear()
            flush_deferred(B, NCH)
            while pending:
                pending.popleft()[3]()
            while proj_q:
                proj_q.popleft()[1]()
    if split:
        _split_waits(nc)
    return nc


# ---------------------------------------------------------------- host side


def make_tables(T):
    inv_freq = 1.0 / (ROPE_BASE ** (np.arange(0, HD, 2, dtype=np.float32) / HD))
    pos = np.arange(T, dtype=np.float32)
    freqs = pos[:, None] * inv_freq[None, :]  # [T, 32]
    cos = np.cos(freqs).astype(np.float32)  # [T, 32] (same for both halves)
    sin = np.sin(freqs).astype(np.float32)
    cosT64 = np.concatenate([cos.T, cos.T], axis=0)  # [64, T]
    sinT64 = np.concatenate([-sin.T, sin.T], axis=0)  # sign-baked rotate_half
    cosT = np.concatenate([cosT64, cosT64], axis=0).copy()  # [128, T] two heads
    sinT = np.concatenate([sinT64, sinT64], axis=0).copy()
    return cosT, sinT


def make_perm():
    # perm[k, m] = 1 iff m == (k+32) % 64 within each 64-row head block
    p = np.zeros((128, 128), dtype=np.float32)
    for hb in range(2):
        for k in range(HD):
            p[hb * HD + k, hb * HD + (k + 32) % HD] = 1.0
    return p


def make_mask4():
    # mask4[p, v*512 + f] = 1.0 if v*128 + p <= f else 0.0
    m = np.zeros((128, 4 * 512), dtype=np.float32)
    p = np.arange(128)[:, None]
    f = np.arange(512)[None, :]
    for v in range(4):
        m[:, v * 512 : (v + 1) * 512] = (v * 128 + p <= f).astype(np.float32)
    return m


def make_in_maps(x, W_qkv, b_qkv, W_proj, n_cores):
    B, T, C = x.shape
    import ml_dtypes

    xT = np.ascontiguousarray(x.reshape(B * T, C).T.astype(ml_dtypes.bfloat16))
    cosT, sinT = make_tables(T)
    mask4 = make_mask4()
    perm = make_perm()
    in_maps = []
    for c in range(n_cores):
        h0 = 2 * c * HD  # first head's column offset (2 heads per core)
        sl = slice(h0, h0 + 128)
        in_maps.append(
            {
                "xT": xT,
                "wq": np.ascontiguousarray(W_qkv[:, sl].astype(ml_dtypes.bfloat16)),
                "wk": np.ascontiguousarray(
                    W_qkv[:, C:][:, sl].astype(ml_dtypes.bfloat16)
                ),
                "wv": np.ascontiguousarray(
                    W_qkv[:, 2 * C :][:, sl].astype(ml_dtypes.bfloat16)
                ),
                "bq": np.ascontiguousarray(b_qkv[sl].reshape(128, 1)),
                "bk": np.ascontiguousarray(b_qkv[C:][sl].reshape(128, 1)),
                "bv": np.ascontiguousarray(b_qkv[2 * C :][sl].reshape(128, 1)),
                "wp": np.ascontiguousarray(W_proj[sl, :].astype(ml_dtypes.bfloat16)),
                "cosT": np.ascontiguousarray(cosT.astype(ml_dtypes.bfloat16)),
                "sinT": np.ascontiguousarray(sinT.astype(ml_dtypes.bfloat16)),
                "perm": perm.astype(ml_dtypes.bfloat16),
                "mask4": mask4.astype(ml_dtypes.bfloat16),
            }
        )
    return in_maps


_NC_CACHE = {}


def _get_nc(B, T, C):
    key = (B, T, C)
    if key not in _NC_CACHE:
        _NC_CACHE[key] = build_nc(B, T, C)
    return _NC_CACHE[key]


def kernel(x, W_qkv, b_qkv, W_proj, b_proj):
    from concourse.bass_utils import run_bass_kernel_spmd

    x = np.asarray(x, dtype=np.float32)
    W_qkv = np.asarray(W_qkv, dtype=np.float32)
    b_qkv = np.asarray(b_qkv, dtype=np.float32)
    W_proj = np.asarray(W_proj, dtype=np.float32)
    b_proj = np.asarray(b_proj, dtype=np.float32)
    B, T, C = x.shape
    n_cores = 8
    nc = _get_nc(B, T, C)
    in_maps = make_in_maps(x, W_qkv, b_qkv, W_proj, n_cores)
    res = run_bass_kernel_spmd(nc, in_maps, core_ids=list(range(n_cores)))
    out = np.zeros((B * T, C), dtype=np.float32)
    for r in res.results:
        out += r["outp"].astype(np.float32)
    out += b_proj[None, :]
    return out.reshape(B, T, C)


# revision 31
# speedup vs baseline: 1.0342x; 1.0093x over previous
"""Causal self-attention (RoPE) Trainium2 kernel, 8-way head-parallel.

Sharding: each of the 8 cores computes 2 of the 16 heads for all 4 batches
(tensor parallel over heads: W_qkv column-split, W_proj row-split). Host
pre-transposes x -> xT [C, B*T], slices per-core weights, and sum-reduces the
8 partial projection outputs (+ b_proj) — the standard row-parallel TP reduce.

Per-core dataflow (bf16 storage/matmuls, fp32 PSUM):
  qkvT = W_slice.T @ xT            [feat, tok] PSUM, bias added on ACT evac
  RoPE on qT,kT                    (rotate-half via permutation matmul on PE)
  v: PE-transpose vT -> vc tiles   [tok, 2*(HD+1)] (+ ones col per head)
  per (b, h, i-chunk, jg of 2 j-tiles):
      S^T = kT_jt.T @ qT_ichunk    (j on partitions)
      P^T = exp(S^T/8) (ACT), causal mask on diagonal tiles (mult, DVE)
      [yT_h | denom] += vc_h.T @ P^T   accumulated over j-tiles in PSUM
  yT_h *= recip(denom) (approx recip, bf16 bcast), out = sum_h yT_h.T @ Wp_h

The emission schedule software-pipelines at j-group granularity: between a
j-group's S matmuls and the PREVIOUS group's AV matmuls we pop one "filler"
unit (a QKV weight-stream for batch b+1 or a projection half for an earlier
chunk) so the PE stream stays dense while ACT computes exp.
"""

from collections import deque

import numpy as np

import concourse.bass as bass
import concourse.mybir as mybir
import concourse.tile as tile

F32 = mybir.dt.float32
BF16 = mybir.dt.bfloat16
AF = mybir.ActivationFunctionType
OP = mybir.AluOpType

# ---------------------------------------------------------------- tile patch
# This walrus build rejects >1 embedded sync-wait on sync-engine CTRL
# instructions; Tile's tail drain embeds one wait per outstanding semaphore.
# Split them across NOPs (1 wait each) before the drain.


def _patched_drain_and_barrier(self, tick_clock, wait_clock):
    from concourse.tile import ScopedClock

    nc = self.nc
    probe = nc.sync.nop(nofuse=True)
    wait_clock.add_sem_waits(probe.ins, ScopedClock({None: tick_clock.global_clock}))
    si = probe.ins.sync_info
    waits = list(si.on_wait) if si is not None and si.on_wait else []
    if len(waits) > 1:
        si.on_wait = waits[:1]
        for w in waits[1:]:
            nop = nc.sync.nop(nofuse=True)
            nsi = nop.ins.sync_info
            if nsi is None:
                nop.ins.sync_info = mybir.SyncInfo(on_wait=[w], on_update=[])
            else:
                nsi.on_wait = [w]
    nc.sync.drain()
    nc.all_engine_barrier()
    assert self.sems is not None
    popped = nc._tile_sem_poison_stack.pop()
    assert popped is self._sem_poison
    # chunk the sem clears: the range-encoded gpsimd drain (dma_reset) in this
    # walrus build rejects wide semaphore ranges ("ISA wrong length")
    sems = sorted(
        s.num if hasattr(s, "num") else s for s in self.sems.allocated().values()
    )
    for i in range(0, len(sems), 16):
        nc.clear_and_free_semaphores(sems[i : i + 16])
    nc.all_engine_barrier()


tile.TileContext._drain_and_barrier = _patched_drain_and_barrier


def _split_waits(nc):
    """Hoist all but one sync-wait per instruction onto same-engine NOPs
    (this walrus codegen supports a single embedded wait per instruction)."""
    n = 0
    for f in nc.m.functions:
        for blk in f.blocks:
            out = []
            changed = False
            for ins in blk.instructions:
                si = ins.sync_info
                if si is not None and si.on_wait and len(si.on_wait) > 1:
                    waits = list(si.on_wait)
                    for w in waits[:-1]:
                        n += 1
                        out.append(
                            mybir.InstNoOp(
                                name=f"wsplit{n}",
                                engine=ins.engine,
                                sync_info=mybir.SyncInfo(on_wait=[w], on_update=[]),
                                bass_nofuse=True,
                            )
                        )
                    si.on_wait = waits[-1:]
                    changed = True
                out.append(ins)
            if changed:
                blk.instructions = out
    return n


# ------------------------------------------------------------------- builder

HD = 64  # head dim (fixed)
ROPE_BASE = 10000.0


def build_nc(B, T, C, split=True):
    """One core's program: 2 heads x B batches. T % 512 == 0, C % 128 == 0."""
    assert T % 512 == 0 and C % 128 == 0
    TOK = B * T
    KC = C // 128   # contraction chunks for QKV
    NCH = T // 512  # i-chunks per batch
    NJT = T // 128  # j-tiles per batch
    FW = 128        # qkv feature width per tensor (2 heads * 64)
    HALF = NCH // 2

    nc = bass.Bass()
    xT = nc.dram_tensor("xT", [C, TOK], BF16, kind="ExternalInput")
    wq = nc.dram_tensor("wq", [C, FW], BF16, kind="ExternalInput")
    wk = nc.dram_tensor("wk", [C, FW], BF16, kind="ExternalInput")
    wv = nc.dram_tensor("wv", [C, FW], BF16, kind="ExternalInput")
    bq = nc.dram_tensor("bq", [FW, 1], F32, kind="ExternalInput")
    bk = nc.dram_tensor("bk", [FW, 1], F32, kind="ExternalInput")
    bv = nc.dram_tensor("bv", [FW, 1], F32, kind="ExternalInput")
    wp = nc.dram_tensor("wp", [FW, C], BF16, kind="ExternalInput")
    cosT = nc.dram_tensor("cosT", [FW, T], BF16, kind="ExternalInput")
    sinT = nc.dram_tensor("sinT", [FW, T], BF16, kind="ExternalInput")
    perm = nc.dram_tensor("perm", [128, 128], BF16, kind="ExternalInput")
    mask4 = nc.dram_tensor("mask4", [128, 4 * 512], BF16, kind="ExternalInput")
    outp = nc.dram_tensor("outp", [TOK, C], BF16, kind="ExternalOutput")

    xT_r = xT[:, :].rearrange("(a p) t -> p a t", p=128)  # [128, KC, TOK]

    with tile.TileContext(nc) as tc:
        with (
            tc.tile_pool(name="const", bufs=1) as cpool,
            tc.tile_pool(name="xt", bufs=4) as xpool,
            tc.tile_pool(name="qk", bufs=2) as qkpool,
            tc.tile_pool(name="vv", bufs=2) as vpool,
            tc.tile_pool(name="yy", bufs=2) as ypool,
            tc.tile_pool(name="small", bufs=3) as spool,
            tc.tile_pool(name="den", bufs=2) as dpool,
            tc.tile_pool(name="pt", bufs=6) as ptpool,
            tc.tile_pool(name="outs", bufs=4) as opool,
            tc.tile_pool(name="dram", bufs=2, space="DRAM") as drampool,
            tc.tile_pool(name="ps_mm", bufs=2, space="PSUM") as ps_mm,
            tc.tile_pool(name="ps_s", bufs=2, space="PSUM") as ps_s,
            tc.tile_pool(name="ps_y", bufs=1, space="PSUM") as ps_y,
        ):
            # ---- constants (priority order: first QKV chunk's deps first) ----
            w_sb = {}
            b_sb = {}

            def load_w(name, dram, dt=BF16):
                t = cpool.tile([128, KC, FW], dt, tag=name)
                for g in range(0, KC, KC // 2):
                    nc.sync.dma_start(
                        t[:, g : g + KC // 2, :],
                        dram[:, :].rearrange("(a p) f -> p a f", p=128)[
                            :, g : g + KC // 2, :
                        ],
                    )
                w_sb[name] = t

            def load_b(bname, bias_d):
                bt = cpool.tile([FW, 1], F32, tag=bname)
                nc.sync.dma_start(bt[:, :], bias_d[:, :])
                b_sb[bname] = bt

            perm_sb = cpool.tile([128, 128], BF16, tag="perm")
            cos_sb = cpool.tile([FW, T], BF16, tag="cos")
            sin_sb = cpool.tile([FW, T], BF16, tag="sin")
            mask_sb = cpool.tile([128, 4 * 512], BF16, tag="mask")
            wp2_sb = cpool.tile([FW, C], BF16, tag="wp2")

            def late_consts():
                for g in range(0, T, T // 2):
                    nc.sync.dma_start(
                        cos_sb[:, g : g + T // 2], cosT[:, g : g + T // 2]
                    )
                    nc.sync.dma_start(
                        sin_sb[:, g : g + T // 2], sinT[:, g : g + T // 2]
                    )
                nc.sync.dma_start(mask_sb[:, :], mask4[:, :])
                nc.sync.dma_start(wp2_sb[:, :], wp[:, :])

            ident = cpool.tile([128, 128], BF16, tag="ident")
            from concourse.masks import make_identity

            make_identity(nc, ident[:, :])
            ones_raw = cpool.tile([128, 128], F32, tag="ones_raw")
            nc.vector.memset(ones_raw[:, :], 1.0)

            state = {}
            # PE/DVE ops deferred two units so PE never waits on ACT evacs
            defq = deque()  # (due_unit, b, cn, fn)
            unit_ctr = [0]

            def defer(b, cn, fn):
                defq.append((unit_ctr[0] + 2, b, cn, fn))

            def run_deferred():
                while defq and defq[0][0] <= unit_ctr[0]:
                    defq.popleft()[3]()

            def flush_deferred(b, ic):
                while defq and (defq[0][1] < b or
                                (defq[0][1] == b and defq[0][2] <= ic)):
                    defq.popleft()[3]()

            def alloc_qkv(b):
                st = state.setdefault(b, {})
                st["qT"] = qkpool.tile([FW, T], BF16, tag="qT", name=f"qT{b}")
                # per-head K tiles, other head's rows zeroed: keeps the S
                # stationary a full 128x128 tile (no PE tile-config switch)
                st["kT0"] = qkpool.tile([FW, T], BF16, tag="kT0", name=f"kT0{b}")
                st["kT1"] = qkpool.tile([FW, T], BF16, tag="kT1", name=f"kT1{b}")
                nc.vector.memset(st["kT0"][HD:FW, :], 0.0)
                nc.vector.memset(st["kT1"][0:HD, :], 0.0)
                # v combined: per j-tile and head, [v_h (64) | ones | zeros(63)]
                # so the AV stationary is also a full 128x128 tile
                st["vc"] = vpool.tile([128, NJT, 256], BF16, tag="vc",
                                      name=f"vc{b}")
                for h in range(2):
                    nc.vector.tensor_copy(
                        st["vc"][:, :, h * 128 + HD],
                        ones_raw[:, 0:1].broadcast_to([128, NJT]),
                    )
                    nc.vector.memset(
                        st["vc"][:, :, h * 128 + HD + 1 : h * 128 + 128], 0.0
                    )

            def fetch_xt(b, fcn):
                st = state.setdefault(b, {})
                if ("xt", fcn) in st or fcn >= NCH:
                    return
                fx = xpool.tile([128, KC, 512], BF16, tag="xt",
                                name=f"xt{b}_{fcn}")
                st[("xt", fcn)] = fx
                fs0 = b * T + fcn * 512
                for g in range(0, KC, 2):
                    nc.sync.dma_start(
                        fx[:, g : g + 2, :],
                        xT_r[:, g : g + 2, fs0 : fs0 + 512],
                    )

            def unit_qkv(b, cn, name):
                def fn():
                    unit_ctr[0] += 1
                    run_deferred()
                    if "qT" not in state.setdefault(b, {}):
                        alloc_qkv(b)
                    st = state[b]
                    tok0 = b * T
                    ts0 = cn * 512
                    if name == "wq":
                        fetch_xt(b, cn)
                        fetch_xt(b, cn + 1)
                    xt = st[("xt", cn)]
                    ps = ps_mm.tile([128, 512], F32, tag="mm",
                                    name=f"qkvps{b}_{cn}_{name}")
                    for kc in range(KC):
                        nc.tensor.matmul(
                            ps[:, :],
                            lhsT=w_sb[name][:, kc, :],
                            rhs=xt[:, kc, :],
                            start=(kc == 0),
                            stop=(kc == KC - 1),
                        )
                    if name == "wv":
                        st.pop(("xt", cn))
                        vch = spool.tile([128, 512], BF16, tag="vch",
                                         name=f"vch{b}_{cn}")
                        nc.scalar.activation(
                            vch[:, :], ps[:, :], AF.Identity, bias=b_sb["bv"][:, :]
                        )

                        def dtrans():
                            vc = state[b]["vc"]
                            for qd in range(4):
                                pst = ps_mm.tile([128, 128], BF16, tag="mm",
                                                 name=f"pst{b}_{cn}_{qd}")
                                nc.tensor.transpose(
                                    pst[:, :],
                                    vch[:, qd * 128 : qd * 128 + 128],
                                    ident[:, :],
                                )
                                jt = cn * 4 + qd
                                nc.vector.tensor_copy(
                                    vc[:, jt, :].rearrange(
                                        "p (a c) -> p a c", a=2
                                    )[:, :, 0:HD],
                                    pst[:, :].rearrange("p (a c) -> p a c", a=2),
                                )

                        defer(b, cn, dtrans)
                    else:
                        if name == "wq":
                            dch = st["qT"][:, ts0 : ts0 + 512]
                            bias = b_sb["bq"]
                        else:
                            ktmp = spool.tile([128, 512], BF16, tag="ktmp",
                                              name=f"ktmp{b}_{cn}")
                            dch = ktmp[:, :]
                            bias = b_sb["bk"]
                        nc.scalar.activation(dch, ps[:, :], AF.Identity, bias=bias[:, :])

                        def drope():
                            swp = ps_mm.tile([128, 512], F32, tag="mm",
                                             name=f"swp{b}_{cn}_{name}")
                            nc.tensor.matmul(
                                swp[:, :], lhsT=perm_sb[:, :], rhs=dch,
                                start=True, stop=True,
                            )
                            cc = cos_sb[:, ts0 : ts0 + 512]
                            ss = sin_sb[:, ts0 : ts0 + 512]
                            t1 = spool.tile([128, 512], F32, tag="t1",
                                            name=f"t1{b}_{cn}_{name}")
                            t2 = spool.tile([128, 512], F32, tag="t2",
                                            name=f"t2{b}_{cn}_{name}")
                            eng = nc.vector if name == "wq" else nc.gpsimd
                            eng.tensor_tensor(t1[:, :], dch, cc, op=OP.mult)
                            nc.vector.tensor_tensor(t2[:, :], swp[:, :], ss,
                                                    op=OP.mult)
                            if name == "wq":
                                eng.tensor_tensor(dch, t1[:, :], t2[:, :],
                                                  op=OP.add)
                            else:
                                eng.tensor_tensor(
                                    st["kT0"][0:HD, ts0 : ts0 + 512],
                                    t1[0:HD, :], t2[0:HD, :], op=OP.add,
                                )
                                eng.tensor_tensor(
                                    st["kT1"][HD:FW, ts0 : ts0 + 512],
                                    t1[HD:FW, :], t2[HD:FW, :], op=OP.add,
                                )

                        defer(b, cn, drope)

                return fn

            # ---- filler machinery ----
            pending = deque()   # (b, cn, name, fn) in emission order
            proj_q = deque()    # (b, fn)
            tail_stash = []

            def pop_filler(cur_b):
                if pending and pending[0][0] <= cur_b + 1:
                    pending.popleft()[3]()
                elif proj_q:
                    proj_q.popleft()[1]()

            def force_qkv(b, cn):
                while pending and (
                    pending[0][0] < b
                    or (pending[0][0] == b and pending[0][1] <= cn)
                ):
                    pending.popleft()[3]()

            def drain_proj(max_b):
                while proj_q and proj_q[0][0] <= max_b:
                    proj_q.popleft()[1]()

            # ---- attention ----
            def attn_begin(b, ic):
                st = state[b]
                yps = {
                    h: ps_y.tile([128, 512], F32, tag=f"y{h}",
                                 name=f"yps{b}_{ic}_{h}")
                    for h in range(2)
                }
                st[("yps", ic)] = yps
                if ic == 0:
                    st["yTfull"] = ypool.tile([FW, T], BF16, tag="yTfull",
                                              name=f"yTfull{b}")
                    # den rows: per-ic 8-row blocks at 32-aligned partitions
                    st["den_all"] = dpool.tile([32 * (NCH - 1) + 8, 128], F32,
                                               tag="den", name=f"den{b}")
                    st["rec_all"] = dpool.tile([32 * (NCH - 1) + 8, 128], F32,
                                               tag="rec", name=f"rec{b}")
                    st["rec_bf"] = dpool.tile([32 * (NCH - 1) + 8, 128], BF16,
                                              tag="recbf", name=f"recbf{b}")
                    st["rec_dram"] = drampool.tile([2 * NCH, 512], BF16,
                                                   tag="rec_dram",
                                                   name=f"rec_dram{b}")

            def emit_S(b, ic, jg):
                st = state[b]
                i0 = ic * 512
                njt = 4 * (ic + 1)
                dv0 = 2 * jg - (njt - 4)
                scale = float(1.0 / np.sqrt(HD))
                pt = {}
                for h in range(2):
                    kz = st["kT0"] if h == 0 else st["kT1"]
                    sp = ps_s.tile([128, 1024], F32, tag="s",
                                   name=f"sps{b}_{ic}_{jg}_{h}")
                    for li in range(2):
                        jt = 2 * jg + li
                        off = max(jt - (njt - 4), 0) * 128
                        nc.tensor.matmul(
                            sp[:, li * 512 + off : li * 512 + 512],
                            lhsT=kz[:, jt * 128 : jt * 128 + 128],
                            rhs=st["qT"][:, i0 + off : i0 + 512],
                            start=True,
                            stop=True,
                        )
                    p = ptpool.tile([128, 1024], BF16, tag="pt",
                                    name=f"pt{b}_{ic}_{jg}_{h}")
                    if dv0 < 0:
                        nc.scalar.activation(p[:, :], sp[:, :], AF.Exp,
                                             scale=scale)
                    else:
                        for li in range(2):
                            dv = dv0 + li
                            off = li * 512 + dv * 128
                            nc.scalar.activation(
                                p[:, off : li * 512 + 512],
                                sp[:, off : li * 512 + 512],
                                AF.Exp, scale=scale,
                            )
                    pt[h] = p
                st[("pt", ic, jg)] = pt

            def emit_exp_mask(b, ic, jg):
                st = state[b]
                njt = 4 * (ic + 1)
                dv0 = 2 * jg - (njt - 4)
                pt = st[("pt", ic, jg)]
                if dv0 >= 0:  # mask only the 128-wide diagonal square per tile
                    for h in range(2):
                        for li in range(2):
                            dv = dv0 + li
                            po = li * 512 + dv * 128
                            mo = dv * 512 + dv * 128
                            nc.vector.tensor_tensor(
                                pt[h][:, po : po + 128],
                                pt[h][:, po : po + 128],
                                mask_sb[:, mo : mo + 128],
                                op=OP.mult,
                            )

            def emit_AV(b, ic, jg):
                st = state[b]
                njt = 4 * (ic + 1)
                pt = st.pop(("pt", ic, jg))
                yps = st[("yps", ic)]
                for h in range(2):
                    for li in range(2):
                        jt = 2 * jg + li
                        off = max(jt - (njt - 4), 0) * 128
                        nc.tensor.matmul(
                            yps[h][:, off:512],
                            lhsT=st["vc"][:, jt, h * 128 : h * 128 + 128],
                            rhs=pt[h][:, li * 512 + off : li * 512 + 512],
                            start=(jt == 0),
                            stop=(jt == njt - 1),
                            skip_group_check=True,
                        )

            def attn_end(b, ic):
                st = state[b]
                yps = st.pop(("yps", ic))
                i0 = ic * 512
                yTfull = st["yTfull"]
                if b == B - 1 and ic == NCH - 1:
                    for pic in (ic - 2, ic - 1):
                        if ("rb", pic) in st:
                            prb = st.pop(("rb", pic))
                            pi0 = pic * 512
                            nc.vector.tensor_tensor(
                                yTfull[:, pi0 : pi0 + 512],
                                yTfull[:, pi0 : pi0 + 512],
                                prb[:, :], op=OP.mult,
                            )
                dent = spool.tile([HD + 1, 1024], F32, tag="dent",
                                  name=f"dent{b}_{ic}")
                base = ic * 32
                for h in range(2):
                    nc.vector.tensor_copy(
                        dent[HD : HD + 1, h * 512 : h * 512 + 512],
                        yps[h][HD : HD + 1, :],
                    )
                    b4 = base + h * 4
                    nc.sync.dma_start(
                        st["den_all"][b4 : b4 + 4, :],
                        dent[HD : HD + 1, h * 512 : h * 512 + 512],
                    )
                nc.vector.reciprocal(
                    st["rec_all"][base : base + 8, :],
                    st["den_all"][base : base + 8, :],
                )
                nc.vector.tensor_copy(
                    st["rec_bf"][base : base + 8, :],
                    st["rec_all"][base : base + 8, :],
                )
                ro = ic * 2
                nc.sync.dma_start(
                    st["rec_dram"][ro : ro + 2, :],
                    st["rec_bf"][base : base + 8, :],
                )
                rb = spool.tile([128, 512], BF16, tag="rb", name=f"rb{b}_{ic}")
                nc.sync.dma_start(
                    rb[0:HD, :],
                    st["rec_dram"][ro : ro + 1, :].broadcast_to([HD, 512]),
                )
                nc.sync.dma_start(
                    rb[HD:128, :],
                    st["rec_dram"][ro + 1 : ro + 2, :].broadcast_to([HD, 512]),
                )
                nc.vector.tensor_copy(yTfull[0:HD, i0 : i0 + 512], yps[0][0:HD, :])
                ytmp = spool.tile([HD, 512], BF16, tag="ytmp", name=f"ytmp{b}_{ic}")
                nc.vector.tensor_copy(ytmp[:, :], yps[1][0:HD, :])
                nc.sync.dma_start(yTfull[HD:FW, i0 : i0 + 512], ytmp[:, :])
                st[("rb", ic)] = rb

            def unit_proj(b, ic, tt):
                def fn():
                    run_deferred()
                    st = state[b]
                    tok0 = b * T
                    i0 = ic * 512
                    yTfull = st["yTfull"]
                    if tt == 0 and ("rb", ic) in st:
                        rb = st.pop(("rb", ic))
                        nc.vector.tensor_tensor(
                            yTfull[:, i0 : i0 + 512], yTfull[:, i0 : i0 + 512],
                            rb[:, :], op=OP.mult,
                        )
                    if True:
                        tr0 = i0 + tt * 128
                        for fc in range(C // 512):
                            pp = ps_mm.tile([128, 512], F32, tag="mm",
                                            name=f"pp{b}_{ic}_{tt}_{fc}")
                            nc.tensor.matmul(
                                pp[:, :],
                                lhsT=yTfull[:, tr0 : tr0 + 128],
                                rhs=wp2_sb[:, fc * 512 : fc * 512 + 512],
                                start=True,
                                stop=True,
                            )
                            ot = opool.tile([128, 512], BF16, tag="ot",
                                            name=f"ot{b}_{ic}_{tt}_{fc}")
                            if (tt * 2 + fc) % 2 == 0:
                                nc.vector.tensor_copy(ot[:, :], pp[:, :])
                            else:
                                nc.scalar.activation(ot[:, :], pp[:, :], AF.Copy)
                            nc.sync.dma_start(
                                outp[tok0 + tr0 : tok0 + tr0 + 128,
                                     fc * 512 : fc * 512 + 512],
                                ot[:, :],
                            )

                return fn

            # ---- schedule ----
            fetch_xt(0, 0)
            load_w("wq", wq)
            load_b("bq", bq)
            load_w("wk", wk)
            load_b("bk", bk)
            fetch_xt(0, 1)
            load_w("wv", wv)
            load_b("bv", bv)
            nc.sync.dma_start(perm_sb[:, :], perm[:, :])
            late_consts()

            for b in range(B):
                for cn in range(NCH):
                    for name in ("wq", "wk", "wv"):
                        pending.append((b, cn, name, unit_qkv(b, cn, name)))

            for b in range(B):
                for ic in range(NCH):
                    force_qkv(b, ic)
                    flush_deferred(b, ic)
                    if ic == 0:
                        drain_proj(b - 2)
                    attn_begin(b, ic)
                    njg = 2 * (ic + 1)
                    prev = None
                    for jg in range(njg):
                        emit_S(b, ic, jg)
                        emit_exp_mask(b, ic, jg)
                        pop_filler(b)
                        if prev is not None:
                            emit_AV(b, ic, prev)
                        prev = jg
                    pop_filler(b)
                    emit_AV(b, ic, prev)
                    attn_end(b, ic)
                    units = [(b, unit_proj(b, ic, tt)) for tt in range(4)]
                    if b == B - 1 and ic in (NCH - 3, NCH - 2):
                        tail_stash.extend(units)
                    else:
                        proj_q.extend(units)
                    if b == B - 1 and ic == NCH - 1:
                        proj_q.extendleft(reversed(tail_stash))
                        tail_stash.clear()
            flush_deferred(B, NCH)
            while pending:
                pending.popleft()[3]()
            while proj_q:
                proj_q.popleft()[1]()
    if split:
        _split_waits(nc)
    return nc


# ---------------------------------------------------------------- host side


def make_tables(T):
    inv_freq = 1.0 / (ROPE_BASE ** (np.arange(0, HD, 2, dtype=np.float32) / HD))
    pos = np.arange(T, dtype=np.float32)
    freqs = pos[:, None] * inv_freq[None, :]  # [T, 32]
    cos = np.cos(freqs).astype(np.float32)  # [T, 32] (same for both halves)
    sin = np.sin(freqs).astype(np.float32)
    cosT64 = np.concatenate([cos.T, cos.T], axis=0)  # [64, T]
    sinT64 = np.concatenate([-sin.T, sin.T], axis=0)  # sign-baked rotate_half
    cosT = np.concatenate([cosT64, cosT64], axis=0).copy()  # [128, T] two heads
    sinT = np.concatenate([sinT64, sinT64], axis=0).copy()
    return cosT, sinT


def make_perm():
    # perm[k, m] = 1 iff m == (k+32) % 64 within each 64-row head block
    p = np.zeros((128, 128), dtype=np.float32)
    for hb in range(2):
        for k in range(HD):
            p[hb * HD + k, hb * HD + (k + 32) % HD] = 1.0
    return p


def make_mask4():
    # mask4[p, v*512 + f] = 1.0 if v*128 + p <= f else 0.0
    m = np.zeros((128, 4 * 512), dtype=np.float32)
    p = np.arange(128)[:, None]
    f = np.arange(512)[None, :]
    for v in range(4):
        m[:, v * 512 : (v + 1) * 512] = (v * 128 + p <= f).astype(np.float32)
    return m


def make_in_maps(x, W_qkv, b_qkv, W_proj, n_cores):
    B, T, C = x.shape
    import ml_dtypes

    xT = np.ascontiguousarray(x.reshape(B * T, C).T.astype(ml_dtypes.bfloat16))
    cosT, sinT = make_tables(T)
    mask4 = make_mask4()
    perm = make_perm()
    in_maps = []
    for c in range(n_cores):
        h0 = 2 * c * HD  # first head's column offset (2 heads per core)
        sl = slice(h0, h0 + 128)
        in_maps.append(
            {
                "xT": xT,
                "wq": np.ascontiguousarray(W_qkv[:, sl].astype(ml_dtypes.bfloat16)),
                "wk": np.ascontiguousarray(
                    W_qkv[:, C:][:, sl].astype(ml_dtypes.bfloat16)
                ),
                "wv": np.ascontiguousarray(
                    W_qkv[:, 2 * C :][:, sl].astype(ml_dtypes.bfloat16)
                ),
                "bq": np.ascontiguousarray(b_qkv[sl].reshape(128, 1)),
                "bk": np.ascontiguousarray(b_qkv[C:][sl].reshape(128, 1)),
                "bv": np.ascontiguousarray(b_qkv[2 * C :][sl].reshape(128, 1)),
                "wp": np.ascontiguousarray(W_proj[sl, :].astype(ml_dtypes.bfloat16)),
                "cosT": np.ascontiguousarray(cosT.astype(ml_dtypes.bfloat16)),
                "sinT": np.ascontiguousarray(sinT.astype(ml_dtypes.bfloat16)),
                "perm": perm.astype(ml_dtypes.bfloat16),
                "mask4": mask4.astype(ml_dtypes.bfloat16),
            }
        )
    return in_maps


_NC_CACHE = {}


def _get_nc(B, T, C):
    key = (B, T, C)
    if key not in _NC_CACHE:
        _NC_CACHE[key] = build_nc(B, T, C)
    return _NC_CACHE[key]


def kernel(x, W_qkv, b_qkv, W_proj, b_proj):
    from concourse.bass_utils import run_bass_kernel_spmd

    x = np.asarray(x, dtype=np.float32)
    W_qkv = np.asarray(W_qkv, dtype=np.float32)
    b_qkv = np.asarray(b_qkv, dtype=np.float32)
    W_proj = np.asarray(W_proj, dtype=np.float32)
    b_proj = np.asarray(b_proj, dtype=np.float32)
    B, T, C = x.shape
    n_cores = 8
    nc = _get_nc(B, T, C)
    in_maps = make_in_maps(x, W_qkv, b_qkv, W_proj, n_cores)
    res = run_bass_kernel_spmd(nc, in_maps, core_ids=list(range(n_cores)))
    out = np.zeros((B * T, C), dtype=np.float32)
    for r in res.results:
        out += r["outp"].astype(np.float32)
    out += b_proj[None, :]
    return out.reshape(B, T, C)


# revision 32
# speedup vs baseline: 1.0783x; 1.0426x over previous
"""Causal self-attention (RoPE) Trainium2 kernel, 8-way head-parallel.

Sharding: each of the 8 cores computes 2 of the 16 heads for all 4 batches
(tensor parallel over heads: W_qkv column-split, W_proj row-split). Host
pre-transposes x -> xT [C, B*T], slices per-core weights, and sum-reduces the
8 partial projection outputs (+ b_proj) — the standard row-parallel TP reduce.

Per-core dataflow (bf16 storage/matmuls, fp32 PSUM):
  qkvT = W_slice.T @ xT            [feat, tok] PSUM, bias added on ACT evac
  RoPE on qT,kT                    (rotate-half via permutation matmul on PE)
  v: PE-transpose vT -> vc tiles   [tok, 2*(HD+1)] (+ ones col per head)
  per (b, h, i-chunk, jg of 2 j-tiles):
      S^T = kT_jt.T @ qT_ichunk    (j on partitions)
      P^T = exp(S^T/8) (ACT), causal mask on diagonal tiles (mult, DVE)
      [yT_h | denom] += vc_h.T @ P^T   accumulated over j-tiles in PSUM
  yT_h *= recip(denom) (approx recip, bf16 bcast), out = sum_h yT_h.T @ Wp_h

The emission schedule software-pipelines at j-group granularity: between a
j-group's S matmuls and the PREVIOUS group's AV matmuls we pop one "filler"
unit (a QKV weight-stream for batch b+1 or a projection half for an earlier
chunk) so the PE stream stays dense while ACT computes exp.
"""

from collections import deque

import numpy as np

import concourse.bass as bass
import concourse.mybir as mybir
import concourse.tile as tile

F32 = mybir.dt.float32
BF16 = mybir.dt.bfloat16
AF = mybir.ActivationFunctionType
OP = mybir.AluOpType

# ---------------------------------------------------------------- tile patch
# This walrus build rejects >1 embedded sync-wait on sync-engine CTRL
# instructions; Tile's tail drain embeds one wait per outstanding semaphore.
# Split them across NOPs (1 wait each) before the drain.


def _patched_drain_and_barrier(self, tick_clock, wait_clock):
    from concourse.tile import ScopedClock

    nc = self.nc
    probe = nc.sync.nop(nofuse=True)
    wait_clock.add_sem_waits(probe.ins, ScopedClock({None: tick_clock.global_clock}))
    si = probe.ins.sync_info
    waits = list(si.on_wait) if si is not None and si.on_wait else []
    if len(waits) > 1:
        si.on_wait = waits[:1]
        for w in waits[1:]:
            nop = nc.sync.nop(nofuse=True)
            nsi = nop.ins.sync_info
            if nsi is None:
                nop.ins.sync_info = mybir.SyncInfo(on_wait=[w], on_update=[])
            else:
                nsi.on_wait = [w]
    nc.sync.drain()
    nc.all_engine_barrier()
    assert self.sems is not None
    popped = nc._tile_sem_poison_stack.pop()
    assert popped is self._sem_poison
    # chunk the sem clears: the range-encoded gpsimd drain (dma_reset) in this
    # walrus build rejects wide semaphore ranges ("ISA wrong length")
    sems = sorted(
        s.num if hasattr(s, "num") else s for s in self.sems.allocated().values()
    )
    for i in range(0, len(sems), 16):
        nc.clear_and_free_semaphores(sems[i : i + 16])
    nc.all_engine_barrier()


tile.TileContext._drain_and_barrier = _patched_drain_and_barrier


def _split_waits(nc):
    """Hoist all but one sync-wait per instruction onto same-engine NOPs
    (this walrus codegen supports a single embedded wait per instruction)."""
    n = 0
    for f in nc.m.functions:
        for blk in f.blocks:
            out = []
            changed = False
            for ins in blk.instructions:
                si = ins.sync_info
                if si is not None and si.on_wait and len(si.on_wait) > 1:
                    waits = list(si.on_wait)
                    for w in waits[:-1]:
                        n += 1
                        out.append(
                            mybir.InstNoOp(
                                name=f"wsplit{n}",
                                engine=ins.engine,
                                sync_info=mybir.SyncInfo(on_wait=[w], on_update=[]),
                                bass_nofuse=True,
                            )
                        )
                    si.on_wait = waits[-1:]
                    changed = True
                out.append(ins)
            if changed:
                blk.instructions = out
    return n


# ------------------------------------------------------------------- builder

HD = 64  # head dim (fixed)
ROPE_BASE = 10000.0


def build_nc(B, T, C, split=True):
    """One core's program: 2 heads x B batches. T % 512 == 0, C % 128 == 0."""
    assert T % 512 == 0 and C % 128 == 0
    TOK = B * T
    KC = C // 128   # contraction chunks for QKV
    NCH = T // 512  # i-chunks per batch
    NJT = T // 128  # j-tiles per batch
    FW = 128        # qkv feature width per tensor (2 heads * 64)
    HALF = NCH // 2

    nc = bass.Bass()
    xT = nc.dram_tensor("xT", [C, TOK], BF16, kind="ExternalInput")
    wq = nc.dram_tensor("wq", [C, FW], BF16, kind="ExternalInput")
    wk = nc.dram_tensor("wk", [C, FW], BF16, kind="ExternalInput")
    wv = nc.dram_tensor("wv", [C, FW], BF16, kind="ExternalInput")
    bq = nc.dram_tensor("bq", [FW, 1], F32, kind="ExternalInput")
    bk = nc.dram_tensor("bk", [FW, 1], F32, kind="ExternalInput")
    bv = nc.dram_tensor("bv", [FW, 1], F32, kind="ExternalInput")
    wp = nc.dram_tensor("wp", [FW, C], BF16, kind="ExternalInput")
    cosT = nc.dram_tensor("cosT", [FW, T], BF16, kind="ExternalInput")
    sinT = nc.dram_tensor("sinT", [FW, T], BF16, kind="ExternalInput")
    perm = nc.dram_tensor("perm", [128, 128], BF16, kind="ExternalInput")
    mask4 = nc.dram_tensor("mask4", [128, 4 * 512], BF16, kind="ExternalInput")
    outp = nc.dram_tensor("outp", [TOK, C], BF16, kind="ExternalOutput")

    xT_r = xT[:, :].rearrange("(a p) t -> p a t", p=128)  # [128, KC, TOK]

    with tile.TileContext(nc) as tc:
        with (
            tc.tile_pool(name="const", bufs=1) as cpool,
            tc.tile_pool(name="xt", bufs=5) as xpool,
            tc.tile_pool(name="qk", bufs=2) as qkpool,
            tc.tile_pool(name="vv", bufs=2) as vpool,
            tc.tile_pool(name="yy", bufs=2) as ypool,
            tc.tile_pool(name="small", bufs=3) as spool,
            tc.tile_pool(name="den", bufs=2) as dpool,
            tc.tile_pool(name="pt", bufs=6) as ptpool,
            tc.tile_pool(name="outs", bufs=8) as opool,
            tc.tile_pool(name="dram", bufs=2, space="DRAM") as drampool,
            tc.tile_pool(name="ps_mm", bufs=2, space="PSUM") as ps_mm,
            tc.tile_pool(name="ps_s", bufs=2, space="PSUM") as ps_s,
            tc.tile_pool(name="ps_y", bufs=1, space="PSUM") as ps_y,
        ):
            # ---- constants (priority order: first QKV chunk's deps first) ----
            w_sb = {}
            b_sb = {}

            def load_w(name, dram, dt=BF16):
                t = cpool.tile([128, KC, FW], dt, tag=name)
                for g in range(0, KC, KC // 2):
                    nc.sync.dma_start(
                        t[:, g : g + KC // 2, :],
                        dram[:, :].rearrange("(a p) f -> p a f", p=128)[
                            :, g : g + KC // 2, :
                        ],
                    )
                w_sb[name] = t

            def load_b(bname, bias_d):
                bt = cpool.tile([FW, 1], F32, tag=bname)
                nc.sync.dma_start(bt[:, :], bias_d[:, :])
                b_sb[bname] = bt

            perm_sb = cpool.tile([128, 128], BF16, tag="perm")
            cos_sb = cpool.tile([FW, T], BF16, tag="cos")
            sin_sb = cpool.tile([FW, T], BF16, tag="sin")
            mask_sb = cpool.tile([128, 4 * 512], BF16, tag="mask")
            wp2_sb = cpool.tile([FW, C], BF16, tag="wp2")

            def late_consts():
                for g in range(0, T, T // 2):
                    nc.sync.dma_start(
                        cos_sb[:, g : g + T // 2], cosT[:, g : g + T // 2]
                    )
                    nc.sync.dma_start(
                        sin_sb[:, g : g + T // 2], sinT[:, g : g + T // 2]
                    )
                nc.sync.dma_start(mask_sb[:, :], mask4[:, :])
                nc.sync.dma_start(wp2_sb[:, :], wp[:, :])

            ident = cpool.tile([128, 128], BF16, tag="ident")
            from concourse.masks import make_identity

            make_identity(nc, ident[:, :])
            ones_raw = cpool.tile([128, 128], F32, tag="ones_raw")
            nc.vector.memset(ones_raw[:, :], 1.0)

            state = {}
            # PE/DVE ops deferred two units so PE never waits on ACT evacs
            defq = deque()  # (due_unit, b, cn, fn)
            unit_ctr = [0]

            def defer(b, cn, fn):
                defq.append((unit_ctr[0] + 2, b, cn, fn))

            def run_deferred():
                while defq and defq[0][0] <= unit_ctr[0]:
                    defq.popleft()[3]()

            def flush_deferred(b, ic):
                while defq and (defq[0][1] < b or
                                (defq[0][1] == b and defq[0][2] <= ic)):
                    defq.popleft()[3]()

            def alloc_qkv(b):
                st = state.setdefault(b, {})
                st["qT"] = qkpool.tile([FW, T], BF16, tag="qT", name=f"qT{b}")
                # per-head K tiles, other head's rows zeroed: keeps the S
                # stationary a full 128x128 tile (no PE tile-config switch)
                st["kT0"] = qkpool.tile([FW, T], BF16, tag="kT0", name=f"kT0{b}")
                st["kT1"] = qkpool.tile([FW, T], BF16, tag="kT1", name=f"kT1{b}")
                nc.vector.memset(st["kT0"][HD:FW, :], 0.0)
                nc.vector.memset(st["kT1"][0:HD, :], 0.0)
                # v combined: per j-tile and head, [v_h (64) | ones | zeros(63)]
                # so the AV stationary is also a full 128x128 tile
                st["vc"] = vpool.tile([128, NJT, 256], BF16, tag="vc",
                                      name=f"vc{b}")
                for h in range(2):
                    nc.vector.tensor_copy(
                        st["vc"][:, :, h * 128 + HD],
                        ones_raw[:, 0:1].broadcast_to([128, NJT]),
                    )
                    nc.vector.memset(
                        st["vc"][:, :, h * 128 + HD + 1 : h * 128 + 128], 0.0
                    )

            def fetch_xt(b, fcn):
                st = state.setdefault(b, {})
                if ("xt", fcn) in st or fcn >= NCH:
                    return
                fx = xpool.tile([128, KC, 512], BF16, tag="xt",
                                name=f"xt{b}_{fcn}")
                st[("xt", fcn)] = fx
                fs0 = b * T + fcn * 512
                for g in range(0, KC, 2):
                    nc.sync.dma_start(
                        fx[:, g : g + 2, :],
                        xT_r[:, g : g + 2, fs0 : fs0 + 512],
                    )

            def unit_qkv(b, cn, name):
                def fn():
                    unit_ctr[0] += 1
                    run_deferred()
                    if "qT" not in state.setdefault(b, {}):
                        alloc_qkv(b)
                    st = state[b]
                    tok0 = b * T
                    ts0 = cn * 512
                    if name == "wq":
                        fetch_xt(b, cn)
                        fetch_xt(b, cn + 1)
                    xt = st[("xt", cn)]
                    ps = ps_mm.tile([128, 512], F32, tag="mm",
                                    name=f"qkvps{b}_{cn}_{name}")
                    for kc in range(KC):
                        nc.tensor.matmul(
                            ps[:, :],
                            lhsT=w_sb[name][:, kc, :],
                            rhs=xt[:, kc, :],
                            start=(kc == 0),
                            stop=(kc == KC - 1),
                        )
                    if name == "wv":
                        st.pop(("xt", cn))
                        vch = spool.tile([128, 512], BF16, tag="vch",
                                         name=f"vch{b}_{cn}")
                        nc.scalar.activation(
                            vch[:, :], ps[:, :], AF.Identity, bias=b_sb["bv"][:, :]
                        )

                        def dtrans():
                            vc = state[b]["vc"]
                            for qd in range(4):
                                pst = ps_mm.tile([128, 128], BF16, tag="mm",
                                                 name=f"pst{b}_{cn}_{qd}")
                                nc.tensor.transpose(
                                    pst[:, :],
                                    vch[:, qd * 128 : qd * 128 + 128],
                                    ident[:, :],
                                )
                                jt = cn * 4 + qd
                                nc.vector.tensor_copy(
                                    vc[:, jt, :].rearrange(
                                        "p (a c) -> p a c", a=2
                                    )[:, :, 0:HD],
                                    pst[:, :].rearrange("p (a c) -> p a c", a=2),
                                )

                        defer(b, cn, dtrans)
                    else:
                        if name == "wq":
                            dch = st["qT"][:, ts0 : ts0 + 512]
                            bias = b_sb["bq"]
                        else:
                            ktmp = spool.tile([128, 512], BF16, tag="ktmp",
                                              name=f"ktmp{b}_{cn}")
                            dch = ktmp[:, :]
                            bias = b_sb["bk"]
                        nc.scalar.activation(dch, ps[:, :], AF.Identity, bias=bias[:, :])

                        def drope():
                            swp = ps_mm.tile([128, 512], F32, tag="mm",
                                             name=f"swp{b}_{cn}_{name}")
                            nc.tensor.matmul(
                                swp[:, :], lhsT=perm_sb[:, :], rhs=dch,
                                start=True, stop=True,
                            )
                            cc = cos_sb[:, ts0 : ts0 + 512]
                            ss = sin_sb[:, ts0 : ts0 + 512]
                            t1 = spool.tile([128, 512], F32, tag="t1",
                                            name=f"t1{b}_{cn}_{name}")
                            t2 = spool.tile([128, 512], F32, tag="t2",
                                            name=f"t2{b}_{cn}_{name}")
                            eng = nc.vector if name == "wq" else nc.gpsimd
                            eng.tensor_tensor(t1[:, :], dch, cc, op=OP.mult)
                            nc.vector.tensor_tensor(t2[:, :], swp[:, :], ss,
                                                    op=OP.mult)
                            if name == "wq":
                                eng.tensor_tensor(dch, t1[:, :], t2[:, :],
                                                  op=OP.add)
                            else:
                                eng.tensor_tensor(
                                    st["kT0"][0:HD, ts0 : ts0 + 512],
                                    t1[0:HD, :], t2[0:HD, :], op=OP.add,
                                )
                                eng.tensor_tensor(
                                    st["kT1"][HD:FW, ts0 : ts0 + 512],
                                    t1[HD:FW, :], t2[HD:FW, :], op=OP.add,
                                )

                        defer(b, cn, drope)

                return fn

            # ---- filler machinery ----
            pending = deque()   # (b, cn, name, fn) in emission order
            proj_q = deque()    # (b, fn)
            tail_stash = []

            def pop_filler(cur_b):
                if pending:
                    bb, cn, _, _ = pending[0]
                    # last batch's late chunks stay queued until its own
                    # attention window (no qkv(b+1) filler exists there)
                    if bb <= cur_b + 1 and (bb < B - 1 or bb == cur_b
                                            or cn <= 1):
                        pending.popleft()[3]()
                        return
                if proj_q:
                    proj_q.popleft()[1]()

            def force_qkv(b, cn):
                while pending and (
                    pending[0][0] < b
                    or (pending[0][0] == b and pending[0][1] <= cn)
                ):
                    pending.popleft()[3]()

            def drain_proj(max_b):
                while proj_q and proj_q[0][0] <= max_b:
                    proj_q.popleft()[1]()

            # ---- attention ----
            def attn_begin(b, ic):
                st = state[b]
                yps = {
                    h: ps_y.tile([128, 512], F32, tag=f"y{h}",
                                 name=f"yps{b}_{ic}_{h}")
                    for h in range(2)
                }
                st[("yps", ic)] = yps
                if ic == 0:
                    st["yTfull"] = ypool.tile([FW, T], BF16, tag="yTfull",
                                              name=f"yTfull{b}")
                    # den rows: per-ic 8-row blocks at 32-aligned partitions
                    st["den_all"] = dpool.tile([32 * (NCH - 1) + 8, 128], F32,
                                               tag="den", name=f"den{b}")
                    st["rec_all"] = dpool.tile([32 * (NCH - 1) + 8, 128], F32,
                                               tag="rec", name=f"rec{b}")
                    st["rec_bf"] = dpool.tile([32 * (NCH - 1) + 8, 128], BF16,
                                              tag="recbf", name=f"recbf{b}")
                    st["rec_dram"] = drampool.tile([2 * NCH, 512], BF16,
                                                   tag="rec_dram",
                                                   name=f"rec_dram{b}")

            def emit_S(b, ic, jg):
                st = state[b]
                i0 = ic * 512
                njt = 4 * (ic + 1)
                dv0 = 2 * jg - (njt - 4)
                scale = float(1.0 / np.sqrt(HD))
                pt = {}
                for h in range(2):
                    kz = st["kT0"] if h == 0 else st["kT1"]
                    sp = ps_s.tile([128, 1024], F32, tag="s",
                                   name=f"sps{b}_{ic}_{jg}_{h}")
                    for li in range(2):
                        jt = 2 * jg + li
                        off = max(jt - (njt - 4), 0) * 128
                        nc.tensor.matmul(
                            sp[:, li * 512 + off : li * 512 + 512],
                            lhsT=kz[:, jt * 128 : jt * 128 + 128],
                            rhs=st["qT"][:, i0 + off : i0 + 512],
                            start=True,
                            stop=True,
                        )
                    p = ptpool.tile([128, 1024], BF16, tag="pt",
                                    name=f"pt{b}_{ic}_{jg}_{h}")
                    if dv0 < 0:
                        nc.scalar.activation(p[:, :], sp[:, :], AF.Exp,
                                             scale=scale)
                    else:
                        for li in range(2):
                            dv = dv0 + li
                            off = li * 512 + dv * 128
                            nc.scalar.activation(
                                p[:, off : li * 512 + 512],
                                sp[:, off : li * 512 + 512],
                                AF.Exp, scale=scale,
                            )
                    pt[h] = p
                st[("pt", ic, jg)] = pt

            def emit_exp_mask(b, ic, jg):
                st = state[b]
                njt = 4 * (ic + 1)
                dv0 = 2 * jg - (njt - 4)
                pt = st[("pt", ic, jg)]
                if dv0 >= 0:  # mask only the 128-wide diagonal square per tile
                    for h in range(2):
                        for li in range(2):
                            dv = dv0 + li
                            po = li * 512 + dv * 128
                            mo = dv * 512 + dv * 128
                            nc.vector.tensor_tensor(
                                pt[h][:, po : po + 128],
                                pt[h][:, po : po + 128],
                                mask_sb[:, mo : mo + 128],
                                op=OP.mult,
                            )

            def emit_AV(b, ic, jg):
                st = state[b]
                njt = 4 * (ic + 1)
                pt = st.pop(("pt", ic, jg))
                yps = st[("yps", ic)]
                for h in range(2):
                    for li in range(2):
                        jt = 2 * jg + li
                        off = max(jt - (njt - 4), 0) * 128
                        nc.tensor.matmul(
                            yps[h][:, off:512],
                            lhsT=st["vc"][:, jt, h * 128 : h * 128 + 128],
                            rhs=pt[h][:, li * 512 + off : li * 512 + 512],
                            start=(jt == 0),
                            stop=(jt == njt - 1),
                            skip_group_check=True,
                        )

            def attn_end(b, ic):
                st = state[b]
                yps = st.pop(("yps", ic))
                i0 = ic * 512
                yTfull = st["yTfull"]
                if b == B - 1 and ic == NCH - 1:
                    for pic in (ic - 2, ic - 1):
                        if ("rb", pic) in st:
                            prb = st.pop(("rb", pic))
                            pi0 = pic * 512
                            nc.vector.tensor_tensor(
                                yTfull[:, pi0 : pi0 + 512],
                                yTfull[:, pi0 : pi0 + 512],
                                prb[:, :], op=OP.mult,
                            )
                dent = spool.tile([HD + 1, 1024], F32, tag="dent",
                                  name=f"dent{b}_{ic}")
                base = ic * 32
                for h in range(2):
                    nc.vector.tensor_copy(
                        dent[HD : HD + 1, h * 512 : h * 512 + 512],
                        yps[h][HD : HD + 1, :],
                    )
                    b4 = base + h * 4
                    nc.sync.dma_start(
                        st["den_all"][b4 : b4 + 4, :],
                        dent[HD : HD + 1, h * 512 : h * 512 + 512],
                    )
                nc.vector.reciprocal(
                    st["rec_all"][base : base + 8, :],
                    st["den_all"][base : base + 8, :],
                )
                nc.vector.tensor_copy(
                    st["rec_bf"][base : base + 8, :],
                    st["rec_all"][base : base + 8, :],
                )
                ro = ic * 2
                nc.sync.dma_start(
                    st["rec_dram"][ro : ro + 2, :],
                    st["rec_bf"][base : base + 8, :],
                )
                rb = spool.tile([128, 512], BF16, tag="rb", name=f"rb{b}_{ic}")
                nc.sync.dma_start(
                    rb[0:HD, :],
                    st["rec_dram"][ro : ro + 1, :].broadcast_to([HD, 512]),
                )
                nc.sync.dma_start(
                    rb[HD:128, :],
                    st["rec_dram"][ro + 1 : ro + 2, :].broadcast_to([HD, 512]),
                )
                nc.vector.tensor_copy(yTfull[0:HD, i0 : i0 + 512], yps[0][0:HD, :])
                ytmp = spool.tile([HD, 512], BF16, tag="ytmp", name=f"ytmp{b}_{ic}")
                nc.vector.tensor_copy(ytmp[:, :], yps[1][0:HD, :])
                nc.sync.dma_start(yTfull[HD:FW, i0 : i0 + 512], ytmp[:, :])
                st[("rb", ic)] = rb

            def unit_proj(b, ic, tt):
                def fn():
                    run_deferred()
                    st = state[b]
                    tok0 = b * T
                    i0 = ic * 512
                    yTfull = st["yTfull"]
                    if tt == 0 and ("rb", ic) in st:
                        rb = st.pop(("rb", ic))
                        nc.vector.tensor_tensor(
                            yTfull[:, i0 : i0 + 512], yTfull[:, i0 : i0 + 512],
                            rb[:, :], op=OP.mult,
                        )
                    if True:
                        tr0 = i0 + tt * 128
                        for fc in range(C // 512):
                            pp = ps_mm.tile([128, 512], F32, tag="mm",
                                            name=f"pp{b}_{ic}_{tt}_{fc}")
                            nc.tensor.matmul(
                                pp[:, :],
                                lhsT=yTfull[:, tr0 : tr0 + 128],
                                rhs=wp2_sb[:, fc * 512 : fc * 512 + 512],
                                start=True,
                                stop=True,
                            )
                            ot = opool.tile([128, 512], BF16, tag="ot",
                                            name=f"ot{b}_{ic}_{tt}_{fc}")
                            if (tt * 2 + fc) % 2 == 0:
                                nc.vector.tensor_copy(ot[:, :], pp[:, :])
                            else:
                                nc.scalar.activation(ot[:, :], pp[:, :], AF.Copy)
                            nc.sync.dma_start(
                                outp[tok0 + tr0 : tok0 + tr0 + 128,
                                     fc * 512 : fc * 512 + 512],
                                ot[:, :],
                            )

                return fn

            # ---- schedule ----
            fetch_xt(0, 0)
            load_w("wq", wq)
            load_b("bq", bq)
            load_w("wk", wk)
            load_b("bk", bk)
            fetch_xt(0, 1)
            load_w("wv", wv)
            load_b("bv", bv)
            nc.sync.dma_start(perm_sb[:, :], perm[:, :])
            late_consts()

            for b in range(B):
                for cn in range(NCH):
                    for name in ("wq", "wk", "wv"):
                        pending.append((b, cn, name, unit_qkv(b, cn, name)))

            for b in range(B):
                for ic in range(NCH):
                    force_qkv(b, ic)
                    flush_deferred(b, ic)
                    if ic == 0:
                        drain_proj(b - 2)
                    attn_begin(b, ic)
                    njg = 2 * (ic + 1)
                    prev = None
                    for jg in range(njg):
                        emit_S(b, ic, jg)
                        emit_exp_mask(b, ic, jg)
                        pop_filler(b)
                        if prev is not None:
                            emit_AV(b, ic, prev)
                        prev = jg
                    pop_filler(b)
                    emit_AV(b, ic, prev)
                    attn_end(b, ic)
                    units = [(b, unit_proj(b, ic, tt)) for tt in range(4)]
                    if b == B - 1 and ic in (NCH - 3, NCH - 2):
                        tail_stash.extend(units)
                    else:
                        proj_q.extend(units)
                    if b == B - 1 and ic == NCH - 1:
                        proj_q.extendleft(reversed(tail_stash))
                        tail_stash.clear()
            flush_deferred(B, NCH)
            while pending:
                pending.popleft()[3]()
            while proj_q:
                proj_q.popleft()[1]()
    if split:
        _split_waits(nc)
    return nc


# ---------------------------------------------------------------- host side


def make_tables(T):
    inv_freq = 1.0 / (ROPE_BASE ** (np.arange(0, HD, 2, dtype=np.float32) / HD))
    pos = np.arange(T, dtype=np.float32)
    freqs = pos[:, None] * inv_freq[None, :]  # [T, 32]
    cos = np.cos(freqs).astype(np.float32)  # [T, 32] (same for both halves)
    sin = np.sin(freqs).astype(np.float32)
    cosT64 = np.concatenate([cos.T, cos.T], axis=0)  # [64, T]
    sinT64 = np.concatenate([-sin.T, sin.T], axis=0)  # sign-baked rotate_half
    cosT = np.concatenate([cosT64, cosT64], axis=0).copy()  # [128, T] two heads
    sinT = np.concatenate([sinT64, sinT64], axis=0).copy()
    return cosT, sinT


def make_perm():
    # perm[k, m] = 1 iff m == (k+32) % 64 within each 64-row head block
    p = np.zeros((128, 128), dtype=np.float32)
    for hb in range(2):
        for k in range(HD):
            p[hb * HD + k, hb * HD + (k + 32) % HD] = 1.0
    return p


def make_mask4():
    # mask4[p, v*512 + f] = 1.0 if v*128 + p <= f else 0.0
    m = np.zeros((128, 4 * 512), dtype=np.float32)
    p = np.arange(128)[:, None]
    f = np.arange(512)[None, :]
    for v in range(4):
        m[:, v * 512 : (v + 1) * 512] = (v * 128 + p <= f).astype(np.float32)
    return m


def make_in_maps(x, W_qkv, b_qkv, W_proj, n_cores):
    B, T, C = x.shape
    import ml_dtypes

    xT = np.ascontiguousarray(x.reshape(B * T, C).T.astype(ml_dtypes.bfloat16))
    cosT, sinT = make_tables(T)
    mask4 = make_mask4()
    perm = make_perm()
    in_maps = []
    for c in range(n_cores):
        h0 = 2 * c * HD  # first head's column offset (2 heads per core)
        sl = slice(h0, h0 + 128)
        in_maps.append(
            {
                "xT": xT,
                "wq": np.ascontiguousarray(W_qkv[:, sl].astype(ml_dtypes.bfloat16)),
                "wk": np.ascontiguousarray(
                    W_qkv[:, C:][:, sl].astype(ml_dtypes.bfloat16)
                ),
                "wv": np.ascontiguousarray(
                    W_qkv[:, 2 * C :][:, sl].astype(ml_dtypes.bfloat16)
                ),
                "bq": np.ascontiguousarray(b_qkv[sl].reshape(128, 1)),
                "bk": np.ascontiguousarray(b_qkv[C:][sl].reshape(128, 1)),
                "bv": np.ascontiguousarray(b_qkv[2 * C :][sl].reshape(128, 1)),
                "wp": np.ascontiguousarray(W_proj[sl, :].astype(ml_dtypes.bfloat16)),
                "cosT": np.ascontiguousarray(cosT.astype(ml_dtypes.bfloat16)),
                "sinT": np.ascontiguousarray(sinT.astype(ml_dtypes.bfloat16)),
                "perm": perm.astype(ml_dtypes.bfloat16),
                "mask4": mask4.astype(ml_dtypes.bfloat16),
            }
        )
    return in_maps


_NC_CACHE = {}


def _get_nc(B, T, C):
    key = (B, T, C)
    if key not in _NC_CACHE:
        _NC_CACHE[key] = build_nc(B, T, C)
    return _NC_CACHE[key]


def kernel(x, W_qkv, b_qkv, W_proj, b_proj):
    from concourse.bass_utils import run_bass_kernel_spmd

    x = np.asarray(x, dtype=np.float32)
    W_qkv = np.asarray(W_qkv, dtype=np.float32)
    b_qkv = np.asarray(b_qkv, dtype=np.float32)
    W_proj = np.asarray(W_proj, dtype=np.float32)
    b_proj = np.asarray(b_proj, dtype=np.float32)
    B, T, C = x.shape
    n_cores = 8
    nc = _get_nc(B, T, C)
    in_maps = make_in_maps(x, W_qkv, b_qkv, W_proj, n_cores)
    res = run_bass_kernel_spmd(nc, in_maps, core_ids=list(range(n_cores)))
    out = np.zeros((B * T, C), dtype=np.float32)
    for r in res.results:
        out += r["outp"].astype(np.float32)
    out += b_proj[None, :]
    return out.reshape(B, T, C)
